# revision 1
# baseline (speedup 1.0000x reference)
"""Trainium2 Bass kernel for nn_GTShapelet (GIN stack + CLS-query MHA).

Self-contained: builds the Bass/Tile program, shards inputs across 8
NeuronCores (data-parallel over destination-node ranges; graphs 4c..4c+3
on core c), runs via run_bass_kernel_spmd, and reassembles the full
[32, 128] output.

Key algorithmic points:
  - Only y[:, -1, :] (the CLS token) is returned by the reference, so the
    attention needs just one query per graph, and that query is
    h-independent (cls_embedding is shared): scores/softmax collapse to a
    [4, 1025] problem per graph.
  - Layer 1 folds embed_table @ W1 into a 1024x256 table T1 so the whole
    layer is gathers from T1: gelu(T1[nids] + sum_e ew*T1[nids[src]] + b1).
  - segment_sum is computed per 64-destination chunk as a sum of PE
    matmuls S_b^T.T @ X_b where X_b are 128 gathered source rows and
    S_b^T[e, d] = ew[e] * (dst_local[e] == d), built on DVE from shipped
    dst_local/ew vectors.
  - Between GIN layers the 8 cores exchange their [4096, 256] bf16 h
    shards with an AllGather so the next layer's gathers see all nodes.
"""

import sys

if "/opt/trn_rl_repo" not in sys.path:
    sys.path.insert(0, "/opt/trn_rl_repo")

import numpy as np
import ml_dtypes  # noqa: F401  (np 'bfloat16' dtype registration)

# ---- problem constants (hardcoded per spec) ----
B, N, E, D = 32, 1024, 524288, 128
H, HD = 4, 32
F2 = 2 * D  # 256
NCORES = 8
NPC = B * N // NCORES          # 4096 nodes per core
GPC = B // NCORES              # 4 graphs per core
CHW = 64                       # dst-chunk width (nodes)
NCH = NPC // CHW               # 64 chunks per core
NB = 9                         # batches per chunk: 8 main + 1 tail
ACAP = 1024                    # main gather slots per chunk (one call)
TAIL = 128                     # tail slots per chunk
CHCAP = ACAP + TAIL            # 1152 edge capacity per chunk
SEG = 8                        # chunks per segment
NSEG = NCH // SEG              # 8 segments per core
SEGSLOTS = SEG * ACAP + SEG * TAIL   # 9216: [8x1024 main | 8x128 tails]
CAP = NSEG * SEGSLOTS          # 73728 slots per core
NBAT = SEG * NB                # 72 batches per segment (64 main + 8 tail)
BF16 = np.dtype('bfloat16')

_prog_cache = {}


def _build_program(variant="hw", phases=4):
    key = (variant, phases)
    if key in _prog_cache:
        return _prog_cache[key]
    import concourse.bacc as bacc
    import concourse.tile as tile
    import concourse.mybir as mybir
    from concourse.library_config import mlp

    dt = mybir.dt
    AF = mybir.ActivationFunctionType
    OP = mybir.AluOpType

    nc = bacc.Bacc("TRN2", target_bir_lowering=False, debug=False,
                   num_devices=(1 if variant == "sim1" else NCORES))

    def din(name, shape, dtype):
        return nc.dram_tensor(name, shape, dtype, kind="ExternalInput")

    t1 = din("t1", [N, F2], dt.bfloat16)
    h0w1own = din("h0w1own", [NPC, F2], dt.bfloat16)
    ct = din("ct", [128, N // 128, NPC], dt.bfloat16)
    idx12 = din("idx12", [128, CAP // 16], dt.int16)
    cnts = din("cnts", [1, NCH + NSEG], dt.int32)
    dstl = din("dstl", [128, NSEG * NBAT], dt.bfloat16)
    eww = din("eww", [128, NSEG * NBAT], dt.bfloat16)
    iota_bd = din("iota_bd", [128, CHW * NBAT], dt.bfloat16)
    i64 = din("i64", [128, 64], dt.bfloat16)
    i128 = din("i128", [128, 128], dt.bfloat16)
    i128f = din("i128f", [128, 128], dt.float32)
    i4 = din("i4", [4, 4], dt.bfloat16)
    w2t = din("w2t", [128, 4 * 128], dt.bfloat16)
    b2 = din("b2", [128, 2], dt.float32)
    w3t = din("w3t", [128, 2 * 128], dt.bfloat16)
    b3 = din("b3", [128, 1], dt.float32)
    b1 = din("b1", [128, 2], dt.float32)
    wk = din("wk", [128, 128], dt.bfloat16)
    bk = din("bk", [128, 1], dt.float32)
    wv = din("wv", [128, 128], dt.bfloat16)
    bv = din("bv", [128, 1], dt.float32)
    qblk = din("qblk", [128, 4], dt.bfloat16)
    vc4 = din("vc4", [4, 128], dt.bfloat16)
    e4 = din("e4", [4, 4], dt.bfloat16)
    msel = din("msel", [128, 4], dt.float32)
    r4 = din("r4", [4, 128], dt.float32)
    ecls = din("ecls", [4, 1], dt.float32)
    eps = din("eps", [4, 1], dt.float32)
    ynb = din("ynb", [128, 1], dt.float32)
    wo = din("wo", [128, 128], dt.bfloat16)
    lng = din("lng", [4, 128], dt.float32)
    lnb = din("lnb", [4, 128], dt.float32)
    y_out = nc.dram_tensor("y", [GPC, D], dt.float32, kind="ExternalOutput")

    with tile.TileContext(nc) as tc:
        nc.gpsimd.load_library(mlp)
        with tc.tile_pool(name="const", bufs=1) as cp, \
             tc.tile_pool(name="res", bufs=1) as rp, \
             tc.tile_pool(name="work", bufs=1) as wp, \
             tc.tile_pool(name="dram", bufs=2, space="DRAM") as dram:

            def cload(ap, shape, dtype):
                t = cp.tile(shape, dtype, name=f"c_{ap.name}")
                nc.sync.dma_start(out=t[:], in_=ap[:])
                return t

            # t1 stays in DRAM (gather source) -- no SBUF copy needed.
            idx12_t = cload(idx12, [128, CAP // 16], dt.int16)
            cnts_t = cload(cnts, [1, NCH + NSEG], dt.int32)
            dstl_t = cload(dstl, [128, NSEG * NBAT], dt.bfloat16)
            eww_t = cload(eww, [128, NSEG * NBAT], dt.bfloat16)
            iota_t = cload(iota_bd, [128, CHW * NBAT], dt.bfloat16)
            i64_t = cload(i64, [128, 64], dt.bfloat16)
            i128_t = cload(i128, [128, 128], dt.bfloat16)
            i128f_t = cload(i128f, [128, 128], dt.float32)
            i4_t = cload(i4, [4, 4], dt.bfloat16)
            w2t_t = cload(w2t, [128, 4 * 128], dt.bfloat16)
            b2_t = cload(b2, [128, 2], dt.float32)
            w3t_t = cload(w3t, [128, 2 * 128], dt.bfloat16)
            b3_t = cload(b3, [128, 1], dt.float32)
            b1_t = cload(b1, [128, 2], dt.float32)
            wk_t = cload(wk, [128, 128], dt.bfloat16)
            bk_t = cload(bk, [128, 1], dt.float32)
            wv_t = cload(wv, [128, 128], dt.bfloat16)
            bv_t = cload(bv, [128, 1], dt.float32)
            qblk_t = cload(qblk, [128, 4], dt.bfloat16)
            vc4_t = cload(vc4, [4, 128], dt.bfloat16)
            e4_t = cload(e4, [4, 4], dt.bfloat16)
            msel_t = cload(msel, [128, 4], dt.float32)
            r4_t = cload(r4, [4, 128], dt.float32)
            ecls_t = cload(ecls, [4, 1], dt.float32)
            eps_t = cload(eps, [4, 1], dt.float32)
            ynb_t = cload(ynb, [128, 1], dt.float32)
            wo_t = cload(wo, [128, 128], dt.bfloat16)
            lng_t = cload(lng, [4, 128], dt.float32)
            lnb_t = cload(lnb, [4, 128], dt.float32)

            # persistent per-layer state (double-buffered by hand)
            hown = [rp.tile([128, NPC // 128, F2], dt.bfloat16, tag=f"hown{i}",
                            name=f"hown{i}") for i in range(2)]
            hT = [rp.tile([128, 2, NPC], dt.bfloat16, tag=f"hT{i}",
                          name=f"hT{i}") for i in range(2)]
            rhsT = rp.tile([128, 2, NPC], dt.bfloat16, tag="rhsT")
            # gather X buffers: fixed rotation, memset once for pad safety
            xbufs = [wp.tile([128, 8, F2], dt.bfloat16, tag=f"x{i}",
                             name=f"xbuf{i}") for i in range(4)]
            tailbufs = [wp.tile([128, SEG, F2], dt.bfloat16, tag=f"tb{i}",
                                name=f"tailbuf{i}") for i in range(2)]
            for tb in tailbufs:
                nc.vector.memset(tb[:], 0)
            for xb in xbufs:
                nc.vector.memset(xb[:], 0)
            sseg = [wp.tile([128, NBAT * CHW], dt.bfloat16, tag=f"sseg{i}",
                            name=f"sseg{i}") for i in range(2)]

            t1sb = rp.tile([128, N // 128, F2], dt.bfloat16, name="t1sb")
            nc.sync.dma_start(out=t1sb[:],
                              in_=t1.rearrange("(kk p) f -> p kk f", p=128))
            ctbufs = [wp.tile([128, N // 128, SEG * CHW], dt.bfloat16,
                              tag=f"ctb{i}", name=f"ctb{i}") for i in range(2)]
            # layer 0's "own" rows stream in from DRAM
            nc.sync.dma_start(
                out=hown[0][:],
                in_=h0w1own.rearrange("(t p) f -> p t f", p=128))

            gather_srcs = [t1]
            with tc.tile_pool(name="gin_ps", bufs=1, space="PSUM") as pp, \
                 tc.tile_pool(name="gin_sb", bufs=1) as gp:
                idx_t = idx12_t
                for l in range(max(0, min(3, phases))):
                    gsrc = gather_srcs[l]
                    own = hown[l % 2]
                    hT_cur = hT[l % 2]
                    for s in range(NSEG):
                        nbat = NBAT
                        if l == 0:
                            ctb = ctbufs[s % 2]
                            nc.sync.dma_start(
                                out=ctb[:],
                                in_=ct[:, :, s * SEG * CHW:(s + 1) * SEG * CHW])
                        else:
                            # selection matrices for chunks [s*SEG, (s+1)*SEG)
                            # layout: element (d, b) at col d*nbat + b so the
                            # last AP dim is stride-1 (DVE 2x eligibility)
                            st = sseg[s % 2]
                            bb0 = s * NBAT
                            # one merged gather for all 8 chunks' tails
                            tb = tailbufs[s % 2]
                            tib = (s * SEGSLOTS + SEG * ACAP) // 16
                            cregt = nc.gpsimd.value_load(
                                cnts_t[0:1, NCH + s:NCH + s + 1])
                            nc.gpsimd.dma_gather(
                                tb[:], gsrc[:], idx_t[:, tib:tib + 64],
                                1024, cregt, F2)
                            sv = st[:].rearrange("p (d b) -> p d b", b=nbat)
                            nc.vector.tensor_tensor(
                                out=sv,
                                in0=dstl_t[:, bb0:bb0 + nbat].unsqueeze(1)
                                    .broadcast_to([128, CHW, nbat]),
                                in1=iota_t[:].rearrange(
                                    "p (d b) -> p d b", b=nbat),
                                op=OP.is_equal)
                            nc.vector.tensor_tensor(
                                out=sv, in0=sv,
                                in1=eww_t[:, bb0:bb0 + nbat].unsqueeze(1)
                                    .broadcast_to([128, CHW, nbat]),
                                op=OP.mult)
                        for kk in range(SEG):
                            k = s * SEG + kk
                            ps = pp.tile([64, F2], dt.float32, tag="seg", bufs=2)
                            if l == 0:
                                for kt in range(N // 128):
                                    nc.tensor.matmul(
                                        out=ps[:],
                                        lhsT=ctb[:, kt, kk * CHW:(kk + 1) * CHW],
                                        rhs=t1sb[:, kt, :],
                                        start=(kt == 0), stop=False)
                            else:
                                xb = xbufs[k % 4]
                                ib = (s * SEGSLOTS + kk * ACAP) // 16
                                crega = nc.gpsimd.value_load(cnts_t[0:1, k:k + 1])
                                nc.gpsimd.dma_gather(
                                    xb[:], gsrc[:], idx_t[:, ib:ib + 64],
                                    1024, crega, F2)
                                svv = st[:].rearrange("p (d b) -> p d b", b=nbat)
                                for bq in range(8):
                                    nc.tensor.matmul(
                                        out=ps[:],
                                        lhsT=svv[:, :, kk * 8 + bq],
                                        rhs=xb[:, bq, :],
                                        start=(bq == 0), stop=False)
                                nc.tensor.matmul(
                                    out=ps[:], lhsT=svv[:, :, 64 + kk],
                                    rhs=tb[:, kk, :], start=False, stop=False)
                            ochunk = own[(k % 2) * 64:(k % 2) * 64 + 64, k // 2, :]
                            nc.tensor.matmul(out=ps[:],
                                             lhsT=i64_t[(k % 2) * 64:(k % 2) * 64 + 64, :],
                                             rhs=ochunk, start=False, stop=True)
                            msb = gp.tile([64, F2], dt.bfloat16, tag="msb", bufs=3)
                            nc.scalar.activation(msb[:], ps[:], AF.Copy)
                            for j in range(2):
                                tp = pp.tile([128, 64], dt.bfloat16, tag="tp", bufs=2)
                                nc.tensor.transpose(
                                    tp[:], msb[:, j * 128:(j + 1) * 128], i64_t[0:64, :])
                                dst_col = slice(k * 64, (k + 1) * 64)
                                if l == 0:
                                    nc.scalar.activation(
                                        hT_cur[:, j, dst_col], tp[:], AF.Gelu,
                                        bias=b1_t[:, j:j + 1])
                                else:
                                    nc.vector.tensor_copy(
                                        out=rhsT[:, j, dst_col], in_=tp[:])
                    if l > 0:
                        # node matmul with W{l+1} + gelu
                        wt, bt = (w2t_t, b2_t) if l == 1 else (w3t_t, b3_t)
                        fouth = 2 if l == 1 else 1
                        for jo in range(fouth):
                            for m in range(NPC // 512):
                                ps2 = pp.tile([128, 512], dt.float32, tag="nm", bufs=2)
                                for ji in range(2):
                                    if l == 1:
                                        wslice = wt[:, (2 * ji + jo) * 128:(2 * ji + jo + 1) * 128]
                                    else:
                                        wslice = wt[:, ji * 128:(ji + 1) * 128]
                                    nc.tensor.matmul(
                                        out=ps2[:], lhsT=wslice,
                                        rhs=rhsT[:, ji, m * 512:(m + 1) * 512],
                                        start=(ji == 0), stop=(ji == 1))
                                nc.scalar.activation(
                                    hT_cur[:, jo, m * 512:(m + 1) * 512],
                                    ps2[:], AF.Gelu, bias=bt[:, jo:jo + 1])
                    if l < 2:
                        # transpose hT -> node-major, then AllGather
                        hon = hown[(l + 1) % 2]
                        for t in range(NPC // 128):
                            for j in range(2):
                                tp2 = pp.tile([128, 128], dt.bfloat16, tag="tp2", bufs=2)
                                nc.tensor.transpose(
                                    tp2[:], hT_cur[:, j, t * 128:(t + 1) * 128],
                                    i128_t[:])
                                nc.vector.tensor_copy(
                                    out=hon[:, t, j * 128:(j + 1) * 128],
                                    in_=tp2[:])
                        agin = dram.tile([NPC, F2], dt.bfloat16, tag="agin")
                        agout = dram.tile([B * N, F2], dt.bfloat16, tag="agout")
                        agv = agin.rearrange("(t p) f -> p t f", p=128)
                        for q in range(4):
                            nc.sync.dma_start(
                                out=agv[:, q * 8:(q + 1) * 8, :],
                                in_=hon[:, q * 8:(q + 1) * 8, :])
                        if variant == "sim1":
                            # local stand-in for AllGather: same HBM write
                            # volume on the receive side
                            for cc in range(NCORES):
                                nc.sync.dma_start(
                                    out=agout[cc * NPC:(cc + 1) * NPC, :],
                                    in_=agin[:])
                        else:
                            nc.gpsimd.collective_compute(
                                "AllGather", OP.bypass,
                                replica_groups=[list(range(NCORES))],
                                ins=[agin.opt()], outs=[agout.opt()])
                        gather_srcs.append(agout)

            # ---------------- attention + layernorm ----------------
            if phases < 4:
                with tc.tile_pool(name="stub", bufs=1) as sp_:
                    zz = sp_.tile([GPC, D], dt.float32, name="zz")
                    nc.vector.memset(zz[:], 0)
                    # depend on the last phase's output so timing is honest
                    dep = (hown[0][0:1, 0, 0:1] if phases <= 0
                           else hT[(min(3, phases) - 1) % 2][0:1, 0, 0:1])
                    nc.vector.tensor_add(out=zz[0:1, 0:1],
                                         in0=dep, in1=zz[0:1, 0:1])
                    nc.sync.dma_start(out=y_out[:], in_=zz[:])
            elif True:
              h3T = hT[2 % 2]  # [128, 2, NPC]; only [:, 0, :] is meaningful
              with tc.tile_pool(name="att_ps", bufs=1, space="PSUM") as ap_, \
                   tc.tile_pool(name="att_sb", bufs=1) as asb:
                 kT = asb.tile([128, NPC], dt.bfloat16, tag="kT")
                 vnm = asb.tile([128, NPC // 128, 128], dt.bfloat16, tag="vnm")
                 for m in range(NPC // 512):
                     psk = ap_.tile([128, 512], dt.float32, tag="pbig", bufs=2)
                     nc.tensor.matmul(out=psk[:], lhsT=wk_t[:],
                                      rhs=h3T[:, 0, m * 512:(m + 1) * 512])
                     nc.vector.tensor_scalar(
                         out=kT[:, m * 512:(m + 1) * 512], in0=psk[:],
                         scalar1=bk_t[:], scalar2=None, op0=OP.add)
                 for t in range(NPC // 128):
                     psv = ap_.tile([128, 128], dt.float32, tag="pbig", bufs=2)
                     nc.tensor.matmul(out=psv[:],
                                      lhsT=h3T[:, 0, t * 128:(t + 1) * 128],
                                      rhs=wv_t[:])
                     nc.vector.tensor_copy(out=vnm[:, t, :], in_=psv[:])
                 ctx_all = asb.tile([128, 4], dt.bfloat16, tag="ctx_all")
                 for g in range(GPC):
                     ssc = ap_.tile([4, 1024], dt.float32, tag="pbig", bufs=2)
                     for hh in range(2):
                         nc.tensor.matmul(
                             out=ssc[:, hh * 512:(hh + 1) * 512], lhsT=qblk_t[:],
                             rhs=kT[:, g * 1024 + hh * 512: g * 1024 + (hh + 1) * 512])
                     expt = asb.tile([4, 1024], dt.bfloat16, tag="expt")
                     sums = asb.tile([4, 1], dt.float32, tag="sums")
                     nc.scalar.activation(expt[:], ssc[:], AF.Exp,
                                          accum_out=sums[:])
                     nc.vector.tensor_add(out=sums[:], in0=sums[:], in1=ecls_t[:])
                     psr = ap_.tile([128, 1], dt.float32, tag="ptiny", bufs=2)
                     nc.tensor.matmul(out=psr[:], lhsT=r4_t[:], rhs=sums[:])
                     rbc = asb.tile([128, 1], dt.float32, tag="rbc")
                     nc.vector.reciprocal(rbc[:], psr[:])
                     psctx = ap_.tile([128, 4], dt.float32, tag="psctx", bufs=1)
                     for t in range(8):
                         pst = ap_.tile([128, 4], dt.bfloat16, tag="ptiny", bufs=2)
                         nc.tensor.transpose(
                             pst[:], expt[:, t * 128:(t + 1) * 128], i4_t[:])
                         ets = asb.tile([128, 4], dt.bfloat16, tag="ets")
                         nc.vector.tensor_copy(out=ets[:], in_=pst[:])
                         nc.tensor.matmul(out=psctx[:],
                                          lhsT=vnm[:, g * 8 + t, :], rhs=ets[:],
                                          start=(t == 0), stop=False)
                     nc.tensor.matmul(out=psctx[:], lhsT=vc4_t[:], rhs=e4_t[:],
                                      start=False, stop=True)
                     tmp4 = asb.tile([128, 4], dt.float32, tag="tmp4")
                     nc.vector.tensor_tensor(out=tmp4[:], in0=psctx[:],
                                             in1=msel_t[:], op=OP.mult)
                     ctxv = asb.tile([128, 1], dt.float32, tag="ctxv")
                     nc.vector.reduce_sum(out=ctxv[:], in_=tmp4[:],
                                          axis=mybir.AxisListType.X)
                     nc.vector.tensor_scalar(out=ctxv[:], in0=ctxv[:],
                                             scalar1=rbc[:], scalar2=bv_t[:],
                                             op0=OP.mult, op1=OP.add)
                     nc.vector.tensor_copy(out=ctx_all[:, g:g + 1], in_=ctxv[:])
                 psao = ap_.tile([128, 4], dt.float32, tag="ptiny", bufs=2)
                 nc.tensor.matmul(out=psao[:], lhsT=wo_t[:], rhs=ctx_all[:])
                 ysb = asb.tile([128, 4], dt.float32, tag="ysb")
                 nc.vector.tensor_scalar(out=ysb[:], in0=psao[:],
                                         scalar1=ynb_t[:], scalar2=None,
                                         op0=OP.add)
                 psy = ap_.tile([4, 128], dt.float32, tag="ptiny", bufs=2)
                 nc.tensor.matmul(out=psy[:], lhsT=ysb[:], rhs=i128f_t[:],
                                  is_transpose=True)
                 yt = asb.tile([4, 128], dt.float32, tag="yt")
                 nc.vector.tensor_copy(out=yt[:], in_=psy[:])
                 mn = asb.tile([4, 1], dt.float32, tag="mn")
                 nc.vector.reduce_sum(out=mn[:], in_=yt[:],
                                      axis=mybir.AxisListType.X)
                 nc.vector.tensor_scalar(out=mn[:], in0=mn[:],
                                         scalar1=1.0 / D, scalar2=None,
                                         op0=OP.mult)
                 xc = asb.tile([4, 128], dt.float32, tag="xc")
                 nc.vector.tensor_scalar(out=xc[:], in0=yt[:], scalar1=mn[:],
                                         scalar2=None, op0=OP.subtract)
                 sq = asb.tile([4, 128], dt.float32, tag="sq")
                 ss = asb.tile([4, 1], dt.float32, tag="ss")
                 nc.scalar.activation(sq[:], xc[:], AF.Square, accum_out=ss[:])
                 sd = asb.tile([4, 1], dt.float32, tag="sd")
                 nc.scalar.activation(sd[:], ss[:], AF.Sqrt, bias=eps_t[:],
                                      scale=1.0 / D)
                 rr = asb.tile([4, 1], dt.float32, tag="rr")
                 nc.vector.reciprocal(rr[:], sd[:])
                 yn = asb.tile([4, 128], dt.float32, tag="yn")
                 nc.vector.tensor_scalar(out=yn[:], in0=xc[:], scalar1=rr[:],
                                         scalar2=None, op0=OP.mult)
                 nc.vector.tensor_tensor(out=yn[:], in0=yn[:], in1=lng_t[:],
                                         op=OP.mult)
                 nc.vector.tensor_tensor(out=yn[:], in0=yn[:], in1=lnb_t[:],
                                         op=OP.add)
                 nc.sync.dma_start(out=y_out[:], in_=yn[:])

    nc.compile()
    _prog_cache[key] = nc
    return nc


def _wrap16(arr):
    """slot i -> [i % 16, i // 16], replicated into partitions 16..31.

    CoreSim's gather ucode reads partitions 0..15; the deployed HW ucode
    reads 16..31 -- fill both so either path sees the indices.
    """
    n = arr.shape[0]
    out = np.zeros((128, n // 16), np.int16)
    w = arr.reshape(n // 16, 16).T.astype(np.int16)
    out[0:16] = w
    out[16:32] = w
    return out


def _host_prep(inputs):
    node_ids = np.asarray(inputs["node_ids"]).astype(np.int64)
    src = np.asarray(inputs["src"]).astype(np.int64)
    dst = np.asarray(inputs["dst"]).astype(np.int64)
    pad_mask = np.asarray(inputs["pad_mask"])
    ew = np.asarray(inputs["edge_weight"]).astype(np.float64)
    embed = np.asarray(inputs["embed_table"]).astype(np.float64)
    W1 = np.asarray(inputs["W1"]).astype(np.float64)
    b1 = np.asarray(inputs["b1"]).astype(np.float32)
    W2 = np.asarray(inputs["W2"]).astype(np.float32)
    b2 = np.asarray(inputs["b2"]).astype(np.float32)
    W3 = np.asarray(inputs["W3"]).astype(np.float32)
    b3 = np.asarray(inputs["b3"]).astype(np.float32)
    ipw = np.asarray(inputs["in_proj_w"]).astype(np.float64)
    ipb = np.asarray(inputs["in_proj_b"]).astype(np.float64)
    ow = np.asarray(inputs["out_w"]).astype(np.float32)
    ob = np.asarray(inputs["out_b"]).astype(np.float32)
    cls = np.asarray(inputs["cls_embedding"]).astype(np.float64).reshape(D)
    ln_g = np.asarray(inputs["ln_g"]).astype(np.float32)
    ln_b = np.asarray(inputs["ln_b"]).astype(np.float32)

    assert not pad_mask.any(), "kernel compiled for all-False pad_mask"

    # ---- shared (replicated) constants ----
    T1 = (embed @ W1).astype(BF16)                       # [1024, 256]
    Wq, Wk, Wv = ipw[:, :D], ipw[:, D:2 * D], ipw[:, 2 * D:]
    bq, bk_, bv_ = ipb[:D], ipb[D:2 * D], ipb[2 * D:]
    q_cls = (cls @ Wq + bq) / np.sqrt(HD)                # [128]
    k_cls = cls @ Wk + bk_
    v_cls = cls @ Wv + bv_
    s_cls = np.array([q_cls[h * HD:(h + 1) * HD] @ k_cls[h * HD:(h + 1) * HD]
                      for h in range(H)])                # [4]
    e_cls = np.exp(s_cls)
    qblk = np.zeros((128, 4), np.float32)
    for h in range(H):
        qblk[h * HD:(h + 1) * HD, h] = q_cls[h * HD:(h + 1) * HD]
    vc4 = np.zeros((4, 128), np.float32)
    for h in range(H):
        vc4[h, h * HD:(h + 1) * HD] = v_cls[h * HD:(h + 1) * HD]
    e4 = np.diag(e_cls).astype(np.float32)
    msel = np.zeros((128, 4), np.float32)
    for h in range(H):
        msel[h * HD:(h + 1) * HD, h] = 1.0
    r4 = np.zeros((4, 128), np.float32)
    for h in range(H):
        r4[h, h * HD:(h + 1) * HD] = 1.0
    w2tiles = np.concatenate(
        [W2[ji * 128:(ji + 1) * 128, jo * 128:(jo + 1) * 128]
         for ji in range(2) for jo in range(2)], axis=1)  # [128, 512]
    w3tiles = np.concatenate(
        [W3[ji * 128:(ji + 1) * 128, :] for ji in range(2)], axis=1)
    shared = {
        "t1": T1,
        "iota_bd": np.tile(np.repeat(np.arange(CHW, dtype=np.float32), NBAT),
                           (128, 1)).astype(BF16),
        "i64": np.vstack([np.eye(64, dtype=np.float32)] * 2).astype(BF16),
        "i128": np.eye(128, dtype=np.float32).astype(BF16),
        "i128f": np.eye(128, dtype=np.float32),
        "i4": np.eye(4, dtype=np.float32).astype(BF16),
        "w2t": w2tiles.astype(BF16),
        "b2": b2.reshape(2, 128).T.copy(),
        "w3t": w3tiles.astype(BF16),
        "b3": b3.reshape(1, 128).T.copy(),
        "b1": b1.astype(np.float32).reshape(2, 128).T.copy(),
        "wk": Wk.astype(BF16),
        "bk": bk_.astype(np.float32).reshape(128, 1),
        "wv": Wv.astype(BF16),
        "bv": bv_.astype(np.float32).reshape(128, 1),
        "qblk": qblk.astype(BF16),
        "vc4": vc4.astype(BF16),
        "e4": e4.astype(BF16),
        "msel": msel,
        "r4": r4,
        "ecls": e_cls.astype(np.float32).reshape(4, 1),
        "eps": np.full((4, 1), 1e-5, np.float32),
        "ynb": (cls + ob).astype(np.float32).reshape(128, 1),
        "wo": ow.astype(BF16),
        "lng": np.tile(ln_g, (4, 1)),
        "lnb": np.tile(ln_b, (4, 1)),
    }

    # ---- per-core edge partitioning ----
    ew32 = ew.astype(np.float32)
    core_of = dst >> 12           # dst // 4096
    in_maps = []
    order_all = np.argsort(dst, kind='stable')
    dst_sorted = dst[order_all]
    core_starts = np.searchsorted(dst_sorted, np.arange(0, B * N + 1, NPC))
    chunk_starts = np.searchsorted(dst_sorted, np.arange(0, B * N + 1, CHW))
    for c in range(NCORES):
        lo, hi = core_starts[c], core_starts[c + 1]
        eidx = order_all[lo:hi]
        # slot arrays: per segment [8x1024 main | 8x128 tails]
        g_idx12 = np.full(CAP, -1, np.int64)
        sl_dst = np.full(NSEG * NBAT * 128, 100.0, np.float32)
        sl_ew = np.zeros(NSEG * NBAT * 128, np.float32)
        counts = np.zeros(NCH + NSEG, np.int32)
        base_chunk = c * NCH
        for k in range(NCH):
            a = chunk_starts[base_chunk + k] - lo
            bnd = chunk_starts[base_chunk + k + 1] - lo
            cnt = bnd - a
            assert cnt <= CHCAP, f"chunk overflow: {cnt} > {CHCAP}"
            e = eidx[a:bnd]
            s, kk = divmod(k, SEG)
            amain = min(cnt, ACAP)
            em, et = e[:amain], e[amain:]
            s0 = s * SEGSLOTS + kk * ACAP
            t0 = s * SEGSLOTS + SEG * ACAP + kk * TAIL
            g_idx12[s0:s0 + amain] = src[em]
            g_idx12[t0:t0 + cnt - amain] = src[et]
            if cnt - amain < TAIL:
                g_idx12[t0 + max(cnt - amain, 1):t0 + TAIL] = -1
                if cnt - amain == 0:
                    g_idx12[t0] = 0
            # dst_local / ew by batch: main batches kk*8+bq, tail batch 64+kk
            dl = (dst[e] - (c * NPC + k * CHW)).astype(np.float32)
            we = ew32[e]
            bmain0 = s * NBAT * 128 + (kk * 8) * 128
            sl_dst[bmain0:bmain0 + amain] = dl[:amain]
            sl_ew[bmain0:bmain0 + amain] = we[:amain]
            bt0 = s * NBAT * 128 + (64 + kk) * 128
            sl_dst[bt0:bt0 + cnt - amain] = dl[amain:]
            sl_ew[bt0:bt0 + cnt - amain] = we[amain:]
            counts[k] = max(amain, 1)
            if cnt == 0:
                g_idx12[s0] = 0
        # per-segment tail-call counts: up to last real tail slot; interior
        # dummies (idx 0) count as present
        for s in range(NSEG):
            t0 = s * SEGSLOTS + SEG * ACAP
            seg_tail = g_idx12[t0:t0 + SEG * TAIL]
            nz = np.nonzero(seg_tail >= 0)[0]
            if len(nz) == 0:
                g_idx12[t0] = 0
                counts[NCH + s] = 1
            else:
                last = nz[-1]
                # interior -1s must be 0 (gathered dummies)
                interior = seg_tail[:last + 1] < 0
                idxs = np.nonzero(interior)[0]
                g_idx12[t0 + idxs] = 0
                counts[NCH + s] = last + 1
        nids_own = node_ids[c * NPC:(c + 1) * NPC]
        # layer-0 weighted count matrix C[d_local, id] = sum ew over edges
        ids_e = node_ids[src[eidx]]
        dl_e = dst[eidx] - c * NPC
        Cf = np.bincount(dl_e * N + ids_e, weights=ew[eidx],
                         minlength=NPC * N).reshape(NPC, N).astype(np.float32)
        CtT = Cf.T.astype(BF16)          # [N ids, NPC]
        ct_tiles = CtT.reshape(N // 128, 128, NPC).transpose(1, 0, 2).copy()
        m = dict(shared)
        m.update({
            "h0w1own": T1.astype(np.float32)[nids_own].astype(BF16),
            "ct": ct_tiles,
            "idx12": _wrap16(g_idx12),
            "cnts": counts.reshape(1, NCH + NSEG),
            "dstl": sl_dst.reshape(NSEG * NBAT, 128).T.astype(BF16).copy(),
            "eww": sl_ew.reshape(NSEG * NBAT, 128).T.astype(BF16).copy(),
        })
        in_maps.append(m)
    return in_maps


def kernel(**inputs):
    from concourse.bass_utils import run_bass_kernel_spmd
    nc = _build_program()
    in_maps = _host_prep(inputs)
    res = run_bass_kernel_spmd(nc, in_maps, core_ids=list(range(NCORES)))
    y = np.concatenate([res.results[c]["y"] for c in range(NCORES)], axis=0)
    return np.ascontiguousarray(y.astype(np.float32))



# revision 4
# speedup vs baseline: 1.3397x; 1.3397x over previous
"""Trainium2 Bass kernel for nn_GTShapelet (GIN stack + CLS-query MHA).

Self-contained: builds the Bass/Tile program, shards inputs across 8
NeuronCores (data-parallel over destination-node ranges; graphs 4c..4c+3
on core c), runs via run_bass_kernel_spmd, and reassembles the full
[32, 128] output.

Design (vs. the 590us baseline):
  - fp8(e4m3) tables wherever the DMA/PE cost rewards it: gather tables
    (h1, h2), ct count-matrix, T1, W2/W3, selection matrices.  Gather
    rows are 256B (the dma_gather minimum granularity).
  - Edge slots are compacted per 128-dst chunk (128-aligned), and
    dma_gather calls are 1024-index windows decoupled from chunk
    boundaries (the gather ucode caps at 1024 idxs/call).
  - DoubleRow fp8 matmuls (0.5 cyc/row) for the ct and selection
    segment-sum groups and the node matmuls.
  - Selection matrices are built once on DVE (overlapped with layer 1)
    and reused by layers 2 and 3 (identical slotting).
  - The "h + msg" own-row add runs on DVE against the chunk PSUM from
    node-major own tables; biases fold into the own table (L1) or the
    feature-major gelu (L2/L3).
  - The inter-layer AllGather is split into 8 per-segment pieces that
    overlap the producing layer's compute.
  - Attention: K-projection folded into the query on the host
    (scoresT = hT3 @ (Wk qblk); the per-head key-bias constant cancels
    in softmax), scores/exp/V-projection computed node-major per
    segment interleaved with layer 3; no transposes in the tail.
"""

import sys

if "/opt/trn_rl_repo" not in sys.path:
    sys.path.insert(0, "/opt/trn_rl_repo")

import numpy as np
import ml_dtypes

# ---- problem constants (hardcoded per spec) ----
B, N, E, D = 32, 1024, 524288, 128
H, HD = 4, 32
F2 = 2 * D                     # 256
NCORES = 8
NPC = B * N // NCORES          # 4096 nodes per core
GPC = B // NCORES              # 4 graphs per core
CHW = 128                      # dst-chunk width (nodes)
NCH = NPC // CHW               # 32 chunks per core
SEG = 4                        # chunks per segment
NSEG = NCH // SEG              # 8 segments per core
SEGN = SEG * CHW               # 512 nodes per segment
BF16 = ml_dtypes.bfloat16
FP8 = ml_dtypes.float8_e4m3

_prog_cache = {}


def _wrap16(arr):
    """slot i -> [i % 16, i // 16], replicated into partitions 16..31.

    CoreSim's gather ucode reads partitions 0..15; the deployed HW ucode
    reads 16..31 -- fill both so either path sees the indices.
    """
    n = arr.shape[0]
    out = np.zeros((128, n // 16), np.int16)
    w = arr.reshape(n // 16, 16).T.astype(np.int16)
    out[0:16] = w
    out[16:32] = w
    return out


def _host_prep(inputs):
    node_ids = np.asarray(inputs["node_ids"]).astype(np.int64)
    src = np.asarray(inputs["src"]).astype(np.int64)
    dst = np.asarray(inputs["dst"]).astype(np.int64)
    pad_mask = np.asarray(inputs["pad_mask"])
    ew = np.asarray(inputs["edge_weight"]).astype(np.float64)
    embed = np.asarray(inputs["embed_table"]).astype(np.float64)
    W1 = np.asarray(inputs["W1"]).astype(np.float64)
    b1 = np.asarray(inputs["b1"]).astype(np.float32)
    W2 = np.asarray(inputs["W2"]).astype(np.float32)
    b2 = np.asarray(inputs["b2"]).astype(np.float32)
    W3 = np.asarray(inputs["W3"]).astype(np.float32)
    b3 = np.asarray(inputs["b3"]).astype(np.float32)
    ipw = np.asarray(inputs["in_proj_w"]).astype(np.float64)
    ipb = np.asarray(inputs["in_proj_b"]).astype(np.float64)
    ow = np.asarray(inputs["out_w"]).astype(np.float32)
    ob = np.asarray(inputs["out_b"]).astype(np.float32)
    cls = np.asarray(inputs["cls_embedding"]).astype(np.float64).reshape(D)
    ln_g = np.asarray(inputs["ln_g"]).astype(np.float32)
    ln_b = np.asarray(inputs["ln_b"]).astype(np.float32)

    assert not pad_mask.any(), "kernel compiled for all-False pad_mask"

    # ---- shared (replicated) constants ----
    T1 = (embed @ W1).astype(np.float32)                 # [1024, 256]
    t1p = T1.reshape(N // 128, 128, F2).transpose(1, 0, 2).astype(FP8)

    Wq, Wk, Wv = ipw[:, :D], ipw[:, D:2 * D], ipw[:, 2 * D:]
    bq, bk_, bv_ = ipb[:D], ipb[D:2 * D], ipb[2 * D:]
    q_cls = (cls @ Wq + bq) / np.sqrt(HD)                # [128]
    qblk = np.zeros((D, H))
    for h in range(H):
        qblk[h * HD:(h + 1) * HD, h] = q_cls[h * HD:(h + 1) * HD]
    qkf = (Wk @ qblk).astype(np.float32)                 # [128, 4]
    bkq = np.array([bk_ @ qblk[:, h] for h in range(H)])
    k_cls = cls @ Wk + bk_
    s_cls = np.array([q_cls[h * HD:(h + 1) * HD] @ k_cls[h * HD:(h + 1) * HD]
                      for h in range(H)])
    e_cls = np.exp(s_cls - bkq)                          # device scores omit bkq
    v_cls_nb = cls @ Wv                                  # bias added post-softmax
    vc4 = np.zeros((4, 128), np.float32)
    for h in range(H):
        vc4[h, h * HD:(h + 1) * HD] = v_cls_nb[h * HD:(h + 1) * HD]
    e4 = np.diag(e_cls).astype(np.float32)
    msel = np.zeros((128, 4), np.float32)
    r4 = np.zeros((4, 128), np.float32)
    for h in range(H):
        msel[h * HD:(h + 1) * HD, h] = 1.0
        r4[h, h * HD:(h + 1) * HD] = 1.0
    hsel = np.zeros((32, 4), np.float32)
    for j in range(32):
        hsel[j, j % 4] = 1.0

    w2dr = W2.reshape(2, 128, 2, 128).transpose(1, 0, 2, 3).astype(FP8)
    w3dr = W3.reshape(2, 128, 128).transpose(1, 0, 2).astype(FP8)

    # ---- edge slotting (core-uniform: program is SPMD) ----
    ew32 = ew.astype(np.float32)
    order_all = np.argsort(dst, kind='stable')
    dst_sorted = dst[order_all]
    chunk_starts = np.searchsorted(dst_sorted, np.arange(0, B * N + 1, CHW))
    cnt_all = np.diff(chunk_starts).reshape(NCORES, NCH)      # [core, chunk]
    nb_uni = np.maximum(1, -(-cnt_all.max(0) // 128)).astype(np.int64)  # [32]
    B0 = np.concatenate([[0], np.cumsum(nb_uni)]).astype(np.int64)      # [33]
    NBT = int(B0[-1])
    segslots = [int(128 * (B0[SEG * (s + 1)] - B0[SEG * s]))
                for s in range(NSEG)]
    segbase = np.concatenate([[0], np.cumsum(segslots)]).astype(np.int64)
    CAPT = int(segbase[-1])
    SEGBMAX = max(sl // 128 for sl in segslots)

    # agout row permutation: node n -> row (seg<<12 | core<<9 | offset)
    nvec = np.arange(B * N, dtype=np.int64)
    agrow = ((nvec & 4095) >> 9 << 12) | (nvec >> 12 << 9) | (nvec & 511)

    P = dict(nb=tuple(int(x) for x in nb_uni), NBT=NBT, CAPT=CAPT,
             segslots=tuple(segslots), SEGBMAX=SEGBMAX)

    shared = {
        "t1p": t1p,
        "iota128": np.tile(np.arange(CHW, dtype=np.float32),
                           (128, 1)).astype(BF16),
        "w2dr": w2dr.reshape(128, 2 * 2 * 128),
        "w3dr": w3dr.reshape(128, 2 * 128),
        "b2c": b2.reshape(2, 128).T.copy(),
        "b3c": b3.reshape(128, 1).copy(),
        "wvt": Wv.astype(BF16),
        "qkf": qkf.astype(BF16),
        "i128": np.eye(128, dtype=np.float32).astype(BF16),
        "i128f": np.eye(128, dtype=np.float32),
        "ones128": np.ones((128, 1), np.float32).astype(BF16),
        "hsel": hsel,
        "r4": r4,
        "msel": msel,
        "vc4": vc4.astype(BF16),
        "e4": e4.astype(BF16),
        "ecls": e_cls.astype(np.float32).reshape(4, 1),
        "bvt": bv_.astype(np.float32).reshape(128, 1),
        "ynb": (cls + ob).astype(np.float32).reshape(128, 1),
        "wo": ow.astype(BF16),
        "lng": np.tile(ln_g, (4, 1)),
        "lnb": np.tile(ln_b, (4, 1)),
        "eps": np.full((4, 1), 1e-5, np.float32),
        "cnts": (np.arange(9, dtype=np.int32) * 128).reshape(1, 9),
    }

    T1b = T1 + b1[None, :]                                # b1 folded into own rows
    in_maps = []
    for c in range(NCORES):
        g_idx = np.zeros(CAPT, np.int64)
        sl_dst = np.full(NBT * 128, 200.0, np.float32)
        sl_ew = np.zeros(NBT * 128, np.float32)
        for k in range(NCH):
            gk = c * NCH + k
            a, bnd = chunk_starts[gk], chunk_starts[gk + 1]
            cnt = bnd - a
            assert cnt <= 128 * nb_uni[k]
            e = order_all[a:bnd]
            base = 128 * B0[k]
            g_idx[base:base + cnt] = agrow[src[e]]
            sl_dst[base:base + cnt] = (dst[e] - (c * NPC + k * CHW)).astype(np.float32)
            sl_ew[base:base + cnt] = ew32[e]
        eidx = order_all[chunk_starts[c * NCH]:chunk_starts[(c + 1) * NCH]]
        ids_e = node_ids[src[eidx]]
        dl_e = dst[eidx] - c * NPC
        Cf = np.bincount(dl_e * N + ids_e, weights=ew[eidx],
                         minlength=NPC * N).reshape(NPC, N).astype(np.float32)
        # ct[p, k, pr, j, d] = Cf[k*128+d, (2pr+j)*128+p]
        ct = Cf.reshape(NCH, CHW, 8, 128).transpose(3, 0, 2, 1).astype(FP8)
        nids_own = node_ids[c * NPC:(c + 1) * NPC]
        # h0own node-major [128, 32, 256], bias b1 folded in
        h0own = T1b[nids_own].reshape(NCH, CHW, F2).transpose(1, 0, 2).astype(FP8)
        m = dict(shared)
        m.update({
            "ct": np.ascontiguousarray(ct).reshape(128, NCH * 8 * CHW),
            "h0own": np.ascontiguousarray(h0own).reshape(128, NCH * F2),
            "idx12": _wrap16(g_idx),
            "dstl": sl_dst.reshape(NBT, 128).T.astype(BF16).copy(),
            "eww": sl_ew.reshape(NBT, 128).T.astype(BF16).copy(),
        })
        in_maps.append(m)
    return in_maps, P


def _build_program(variant, P):
    key = (variant, P['nb'], P['segslots'])
    if key in _prog_cache:
        return _prog_cache[key]
    import concourse.bacc as bacc
    import concourse.tile as tile
    import concourse.mybir as mybir

    dt = mybir.dt
    AF = mybir.ActivationFunctionType
    OP = mybir.AluOpType
    DR = mybir.MatmulPerfMode.DoubleRow

    nb = P['nb']
    NBT = P['NBT']
    CAPT = P['CAPT']
    segslots = P['segslots']
    SEGBMAX = P['SEGBMAX']
    B0 = [0]
    for x in nb:
        B0.append(B0[-1] + x)
    segbase = [0]
    for sl in segslots:
        segbase.append(segbase[-1] + sl)

    nc = bacc.Bacc("TRN2", target_bir_lowering=False, debug=False,
                   num_devices=(1 if variant == "sim1" else NCORES))

    def din(name, shape, dtype):
        return nc.dram_tensor(name, shape, dtype, kind="ExternalInput")

    t1p = din("t1p", [128, 8, F2], dt.float8e4)
    ct = din("ct", [128, NCH * 8 * CHW], dt.float8e4)
    h0own = din("h0own", [128, NCH * F2], dt.float8e4)
    idx12 = din("idx12", [128, CAPT // 16], dt.int16)
    cnts = din("cnts", [1, 9], dt.int32)
    dstl = din("dstl", [128, NBT], dt.bfloat16)
    eww = din("eww", [128, NBT], dt.bfloat16)
    iota128 = din("iota128", [128, CHW], dt.bfloat16)
    w2dr = din("w2dr", [128, 512], dt.float8e4)
    w3dr = din("w3dr", [128, 256], dt.float8e4)
    b2c = din("b2c", [128, 2], dt.float32)
    b3c = din("b3c", [128, 1], dt.float32)
    wvt = din("wvt", [128, 128], dt.bfloat16)
    qkf = din("qkf", [128, 4], dt.bfloat16)
    i128 = din("i128", [128, 128], dt.bfloat16)
    i128f = din("i128f", [128, 128], dt.float32)
    ones128 = din("ones128", [128, 1], dt.bfloat16)
    hsel = din("hsel", [32, 4], dt.float32)
    r4 = din("r4", [4, 128], dt.float32)
    msel = din("msel", [128, 4], dt.float32)
    vc4 = din("vc4", [4, 128], dt.bfloat16)
    e4 = din("e4", [4, 4], dt.bfloat16)
    ecls = din("ecls", [4, 1], dt.float32)
    bvt = din("bvt", [128, 1], dt.float32)
    ynb = din("ynb", [128, 1], dt.float32)
    wo = din("wo", [128, 128], dt.bfloat16)
    lng = din("lng", [4, 128], dt.float32)
    lnb = din("lnb", [4, 128], dt.float32)
    eps = din("eps", [4, 1], dt.float32)
    y_out = nc.dram_tensor("y", [GPC, D], dt.float32, kind="ExternalOutput")

    with tile.TileContext(nc) as tc:
        from concourse.library_config import mlp
        nc.gpsimd.load_library(mlp)
        with tc.tile_pool(name="const", bufs=1) as cp, \
             tc.tile_pool(name="res", bufs=1) as rp, \
             tc.tile_pool(name="work", bufs=1) as wp, \
             tc.tile_pool(name="ps", bufs=1, space="PSUM") as pp, \
             tc.tile_pool(name="dram", bufs=2, space="DRAM") as dram:

            def cload(ap, shape, dtype):
                t = cp.tile(shape, dtype, name=f"c_{ap.name}")
                nc.sync.dma_start(out=t[:], in_=ap[:])
                return t

            t1p_t = cload(t1p, [128, 8, F2], dt.float8e4)
            h0own_t = cload(h0own, [128, NCH * F2], dt.float8e4)
            h0own_v = h0own_t[:].rearrange("p (k f) -> p k f", f=F2)
            idx_t = cload(idx12, [128, CAPT // 16], dt.int16)
            cnts_t = cload(cnts, [1, 9], dt.int32)
            dstl_t = cload(dstl, [128, NBT], dt.bfloat16)
            eww_t = cload(eww, [128, NBT], dt.bfloat16)
            iota_t = cload(iota128, [128, CHW], dt.bfloat16)
            w2_t = cload(w2dr, [128, 512], dt.float8e4)
            w2_v = w2_t[:].rearrange("p (t jo o) -> p t jo o", t=2, jo=2)
            w3_t = cload(w3dr, [128, 256], dt.float8e4)
            w3_v = w3_t[:].rearrange("p (t o) -> p t o", t=2)
            b2c_t = cload(b2c, [128, 2], dt.float32)
            b3c_t = cload(b3c, [128, 1], dt.float32)
            wvt_t = cload(wvt, [128, 128], dt.bfloat16)
            qkf_t = cload(qkf, [128, 4], dt.bfloat16)
            i128_t = cload(i128, [128, 128], dt.bfloat16)
            i128f_t = cload(i128f, [128, 128], dt.float32)
            ones_t = cload(ones128, [128, 1], dt.bfloat16)
            hsel_t = cload(hsel, [32, 4], dt.float32)
            r4_t = cload(r4, [4, 128], dt.float32)
            msel_t = cload(msel, [128, 4], dt.float32)
            vc4_t = cload(vc4, [4, 128], dt.bfloat16)
            e4_t = cload(e4, [4, 4], dt.bfloat16)
            ecls_t = cload(ecls, [4, 1], dt.float32)
            bvt_t = cload(bvt, [128, 1], dt.float32)
            ynb_t = cload(ynb, [128, 1], dt.float32)
            wo_t = cload(wo, [128, 128], dt.bfloat16)
            lng_t = cload(lng, [4, 128], dt.float32)
            lnb_t = cload(lnb, [4, 128], dt.float32)
            eps_t = cload(eps, [4, 1], dt.float32)

            sel_t = rp.tile([128, NBT, CHW], dt.float8e4, name="sel")
            hon1 = rp.tile([128, NCH, F2], dt.float8e4, name="hon1")
            hon2 = rp.tile([128, NCH, F2], dt.float8e4, name="hon2")
            rhsT = rp.tile([128, 2, NPC], dt.float8e4, name="rhsT")
            hT3 = rp.tile([128, NPC], dt.bfloat16, name="hT3")
            vnm = rp.tile([128, NPC // 128, 128], dt.bfloat16, name="vnm")
            esc = rp.tile([128, 128], dt.bfloat16, name="esc")
            ctx_all = rp.tile([128, 4], dt.bfloat16, name="ctx_all")
            ctbufs = [wp.tile([128, SEG * 8 * CHW], dt.float8e4, tag=f"ctb{i}",
                              name=f"ctb{i}") for i in range(2)]
            gbufs = [wp.tile([128, SEGBMAX, F2], dt.float8e4, tag=f"gb{i}",
                             name=f"gbuf{i}") for i in range(2)]

            agin1 = dram.tile([NPC, F2], dt.float8e4, tag="agin1")
            agout1 = dram.tile([B * N, F2], dt.float8e4, tag="agout1")
            agin2 = dram.tile([NPC, F2], dt.float8e4, tag="agin2")
            agout2 = dram.tile([B * N, F2], dt.float8e4, tag="agout2")

            hon1_v = hon1[:]
            hon2_v = hon2[:]
            state = {"hon_cur": hon1_v}

            def ag_piece(s, agin, agout):
                # send this core's segment-s rows, receive everyone's
                agin_v = agin.rearrange("(g tt p) f -> p g tt f", tt=SEG, p=CHW)
                nc.sync.dma_start(out=agin_v[:, s, :, :],
                                  in_=state["hon_cur"][:, SEG * s:SEG * (s + 1), :])
                if variant == "sim1":
                    for cc in range(NCORES):
                        nc.sync.dma_start(
                            out=agout[s * NPC + cc * SEGN:
                                      s * NPC + (cc + 1) * SEGN, :],
                            in_=agin[s * SEGN:(s + 1) * SEGN, :])
                else:
                    nc.gpsimd.collective_compute(
                        "AllGather", mybir.AluOpType.bypass,
                        replica_groups=[list(range(NCORES))],
                        ins=[agin[s * SEGN:(s + 1) * SEGN, :].opt()],
                        outs=[agout[s * NPC:(s + 1) * NPC, :].opt()])

            # ---------------- layer 1 (ct x T1) ----------------
            nc.sync.dma_start(out=ctbufs[0][:], in_=ct[:, 0:SEG * 8 * CHW])
            for s in range(NSEG):
                if s + 1 < NSEG:
                    nc.sync.dma_start(
                        out=ctbufs[(s + 1) % 2][:],
                        in_=ct[:, (s + 1) * SEG * 8 * CHW:(s + 2) * SEG * 8 * CHW])
                ctb_v = ctbufs[s % 2][:].rearrange(
                    "p (kk pr j d) -> p kk pr j d", pr=4, j=2, d=CHW)
                for kk in range(SEG):
                    k = SEG * s + kk
                    ps = pp.tile([CHW, F2], dt.float32, tag="big", bufs=2)
                    for pr in range(4):
                        nc.tensor.matmul(
                            out=ps[:], lhsT=ctb_v[:, kk, pr, :, :],
                            rhs=t1p_t[:, 2 * pr:2 * pr + 2, :],
                            perf_mode=DR, start=(pr == 0), stop=(pr == 3),
                            skip_group_check=True)
                    msb = wp.tile([CHW, F2], dt.bfloat16, tag="msb", bufs=3)
                    nc.vector.tensor_tensor(out=msb[:], in0=ps[:],
                                            in1=h0own_v[:, k, :], op=OP.add)
                    nc.scalar.activation(hon1_v[:, k, :], msb[:], AF.Gelu)
                # selection matrices for this segment (used by L2+L3)
                nbs = segslots[s] // 128
                b0 = B0[SEG * s]
                nc.vector.tensor_tensor(
                    out=sel_t[:, b0:b0 + nbs, :],
                    in0=dstl_t[:, b0:b0 + nbs].unsqueeze(2)
                        .broadcast_to([128, nbs, CHW]),
                    in1=iota_t[:].unsqueeze(1).broadcast_to([128, nbs, CHW]),
                    op=OP.is_equal)
                nc.vector.tensor_tensor(
                    out=sel_t[:, b0:b0 + nbs, :],
                    in0=sel_t[:, b0:b0 + nbs, :],
                    in1=eww_t[:, b0:b0 + nbs].unsqueeze(2)
                        .broadcast_to([128, nbs, CHW]), op=OP.mult)
                ag_piece(s, agin1, agout1)

            # ---------------- layers 2 and 3 ----------------
            cregs = {}
            for layer in (2, 3):
                table = agout1 if layer == 2 else agout2
                own = hon1_v if layer == 2 else hon2_v
                for s in range(NSEG):
                    gb = gbufs[s % 2]
                    ss_ = segslots[s]
                    ci = 0
                    while ci * 1024 < ss_:
                        w0 = ci * 1024
                        ni = min(1024, ss_ - w0)
                        nbw = ni // 128
                        if nbw not in cregs:
                            cregs[nbw] = nc.gpsimd.value_load(
                                cnts_t[0:1, nbw:nbw + 1])
                        i0 = (segbase[s] + w0) // 16
                        nc.gpsimd.dma_gather(
                            gb[:, w0 // 128:w0 // 128 + nbw, :], table[:],
                            idx_t[:, i0:i0 + ni // 16],
                            ni, cregs[nbw], F2)
                        ci += 1
                    for kk in range(SEG):
                        k = SEG * s + kk
                        nbk = nb[k]
                        b0 = B0[k]
                        g0 = B0[k] - B0[SEG * s]
                        ps = pp.tile([CHW, F2], dt.float32, tag="big", bufs=2)
                        j = 0
                        while j < nbk:
                            if j + 1 < nbk:
                                nc.tensor.matmul(
                                    out=ps[:],
                                    lhsT=sel_t[:, b0 + j:b0 + j + 2, :],
                                    rhs=gb[:, g0 + j:g0 + j + 2, :],
                                    perf_mode=DR, start=(j == 0),
                                    stop=(j + 2 >= nbk), skip_group_check=True)
                                j += 2
                            else:
                                nc.tensor.matmul(
                                    out=ps[:], lhsT=sel_t[:, b0 + j, :],
                                    rhs=gb[:, g0 + j, :],
                                    start=(j == 0), stop=True,
                                    skip_group_check=True)
                                j += 1
                        msb = wp.tile([CHW, F2], dt.bfloat16, tag="msb", bufs=3)
                        nc.vector.tensor_tensor(out=msb[:], in0=ps[:],
                                                in1=own[:, k, :], op=OP.add)
                        for jj in range(2):
                            tp = pp.tile([128, 128], dt.bfloat16, tag="tp", bufs=2)
                            nc.tensor.transpose(
                                tp[:], msb[:, jj * 128:(jj + 1) * 128], i128_t[:])
                            nc.vector.tensor_copy(
                                out=rhsT[:, jj, k * CHW:(k + 1) * CHW], in_=tp[:])
                    # node matmul + gelu for this segment
                    if layer == 2:
                        for jo in range(2):
                            psz = pp.tile([128, SEGN], dt.float32, tag="big",
                                          bufs=2)
                            nc.tensor.matmul(
                                out=psz[:], lhsT=w2_v[:, :, jo, :],
                                rhs=rhsT[:, :, s * SEGN:(s + 1) * SEGN],
                                perf_mode=DR, start=True, stop=True,
                                skip_group_check=True)
                            hf = wp.tile([128, SEGN], dt.bfloat16, tag="hf",
                                         bufs=2)
                            nc.scalar.activation(hf[:], psz[:], AF.Gelu,
                                                 bias=b2c_t[:, jo:jo + 1])
                            for tt in range(SEG):
                                tp2 = pp.tile([128, 128], dt.bfloat16,
                                              tag="tp", bufs=2)
                                nc.tensor.transpose(
                                    tp2[:], hf[:, tt * 128:(tt + 1) * 128],
                                    i128_t[:])
                                nc.vector.tensor_copy(
                                    out=hon2_v[:, SEG * s + tt,
                                               jo * 128:(jo + 1) * 128],
                                    in_=tp2[:])
                        state["hon_cur"] = hon2_v
                        ag_piece(s, agin2, agout2)
                    else:
                        psz = pp.tile([128, SEGN], dt.float32, tag="big", bufs=2)
                        nc.tensor.matmul(
                            out=psz[:], lhsT=w3_v[:, :, :],
                            rhs=rhsT[:, :, s * SEGN:(s + 1) * SEGN],
                            perf_mode=DR, start=True, stop=True,
                            skip_group_check=True)
                        nc.scalar.activation(hT3[:, s * SEGN:(s + 1) * SEGN],
                                             psz[:], AF.Gelu, bias=b3c_t[:, 0:1])
                        # attention tiles for this segment
                        psc = pp.tile([128, 16], dt.float32, tag="psc", bufs=1)
                        for tt in range(4):
                            t = 4 * s + tt
                            nc.tensor.matmul(
                                out=psc[:, tt * 4:(tt + 1) * 4],
                                lhsT=hT3[:, t * 128:(t + 1) * 128], rhs=qkf_t[:],
                                start=True, stop=True)
                            psv = pp.tile([128, 128], dt.float32, tag="att",
                                          bufs=2)
                            nc.tensor.matmul(
                                out=psv[:], lhsT=hT3[:, t * 128:(t + 1) * 128],
                                rhs=wvt_t[:], start=True, stop=True)
                            nc.vector.tensor_copy(out=vnm[:, t, :], in_=psv[:])
                        nc.scalar.activation(esc[:, s * 16:(s + 1) * 16],
                                             psc[:], AF.Exp)
                        if s % 2 == 1:
                            g = s // 2
                            psE = pp.tile([32, 1], dt.float32, tag="att", bufs=2)
                            nc.tensor.matmul(out=psE[:],
                                             lhsT=esc[:, g * 32:(g + 1) * 32],
                                             rhs=ones_t[:], start=True, stop=True)
                            s32 = wp.tile([32, 1], dt.float32, tag="s32", bufs=2)
                            nc.vector.tensor_copy(out=s32[:], in_=psE[:])
                            ps4 = pp.tile([4, 1], dt.float32, tag="att", bufs=2)
                            nc.tensor.matmul(out=ps4[:], lhsT=hsel_t[:],
                                             rhs=s32[:], start=True, stop=True)
                            sums4 = wp.tile([4, 1], dt.float32, tag="sums4",
                                            bufs=2)
                            nc.vector.tensor_tensor(out=sums4[:], in0=ps4[:],
                                                    in1=ecls_t[:], op=OP.add)
                            rr4 = wp.tile([4, 1], dt.float32, tag="rr4", bufs=2)
                            nc.vector.reciprocal(rr4[:], sums4[:])
                            psr = pp.tile([128, 1], dt.float32, tag="att", bufs=2)
                            nc.tensor.matmul(out=psr[:], lhsT=r4_t[:],
                                             rhs=rr4[:], start=True, stop=True)
                            rbc = wp.tile([128, 1], dt.float32, tag="rbc", bufs=2)
                            nc.vector.tensor_copy(out=rbc[:], in_=psr[:])
                            psg = pp.tile([128, 4], dt.float32, tag="att", bufs=2)
                            for t in range(8):
                                nc.tensor.matmul(
                                    out=psg[:], lhsT=vnm[:, 8 * g + t, :],
                                    rhs=esc[:, (8 * g + t) * 4:
                                            (8 * g + t + 1) * 4],
                                    start=(t == 0), stop=False,
                                    skip_group_check=True)
                            nc.tensor.matmul(out=psg[:], lhsT=vc4_t[:],
                                             rhs=e4_t[:], start=False, stop=True,
                                             skip_group_check=True)
                            tmp4 = wp.tile([128, 4], dt.float32, tag="tmp4",
                                           bufs=2)
                            nc.vector.tensor_tensor(out=tmp4[:], in0=psg[:],
                                                    in1=msel_t[:], op=OP.mult)
                            ctxv = wp.tile([128, 1], dt.float32, tag="ctxv",
                                           bufs=2)
                            nc.vector.reduce_sum(out=ctxv[:], in_=tmp4[:],
                                                 axis=mybir.AxisListType.X)
                            nc.vector.tensor_scalar(
                                out=ctx_all[:, g:g + 1], in0=ctxv[:],
                                scalar1=rbc[:], scalar2=bvt_t[:],
                                op0=OP.mult, op1=OP.add)

            # ---------------- output projection + LayerNorm ----------------
            psao = pp.tile([128, 4], dt.float32, tag="att", bufs=2)
            nc.tensor.matmul(out=psao[:], lhsT=wo_t[:], rhs=ctx_all[:],
                             start=True, stop=True)
            ysb = wp.tile([128, 4], dt.float32, tag="ysb")
            nc.vector.tensor_scalar(out=ysb[:], in0=psao[:],
                                    scalar1=ynb_t[:], scalar2=None, op0=OP.add)
            psy = pp.tile([4, 128], dt.float32, tag="att", bufs=2)
            nc.tensor.matmul(out=psy[:], lhsT=ysb[:], rhs=i128f_t[:],
                             is_transpose=True)
            yt = wp.tile([4, 128], dt.float32, tag="yt")
            nc.vector.tensor_copy(out=yt[:], in_=psy[:])
            mn = wp.tile([4, 1], dt.float32, tag="mn")
            nc.vector.reduce_sum(out=mn[:], in_=yt[:], axis=mybir.AxisListType.X)
            nc.vector.tensor_scalar(out=mn[:], in0=mn[:], scalar1=1.0 / D,
                                    scalar2=None, op0=OP.mult)
            xc = wp.tile([4, 128], dt.float32, tag="xc")
            nc.vector.tensor_scalar(out=xc[:], in0=yt[:], scalar1=mn[:],
                                    scalar2=None, op0=OP.subtract)
            sq = wp.tile([4, 128], dt.float32, tag="sq")
            ss = wp.tile([4, 1], dt.float32, tag="ss")
            nc.scalar.activation(sq[:], xc[:], AF.Square, accum_out=ss[:])
            sd = wp.tile([4, 1], dt.float32, tag="sd")
            nc.scalar.activation(sd[:], ss[:], AF.Sqrt, bias=eps_t[:],
                                 scale=1.0 / D)
            rr = wp.tile([4, 1], dt.float32, tag="rr")
            nc.vector.reciprocal(rr[:], sd[:])
            yn = wp.tile([4, 128], dt.float32, tag="yn")
            nc.vector.tensor_scalar(out=yn[:], in0=xc[:], scalar1=rr[:],
                                    scalar2=None, op0=OP.mult)
            nc.vector.tensor_tensor(out=yn[:], in0=yn[:], in1=lng_t[:],
                                    op=OP.mult)
            nc.vector.tensor_tensor(out=yn[:], in0=yn[:], in1=lnb_t[:],
                                    op=OP.add)
            nc.sync.dma_start(out=y_out[:], in_=yn[:])

    nc.compile()
    _prog_cache[key] = nc
    return nc


def kernel(**inputs):
    from concourse.bass_utils import run_bass_kernel_spmd
    in_maps, P = _host_prep(inputs)
    nc = _build_program("hw", P)
    res = run_bass_kernel_spmd(nc, in_maps, core_ids=list(range(NCORES)))
    y = np.concatenate([res.results[c]["y"] for c in range(NCORES)], axis=0)
    return np.ascontiguousarray(y.astype(np.float32))


# revision 5
# speedup vs baseline: 1.6676x; 1.2447x over previous
"""Trainium2 Bass kernel for nn_GTShapelet (GIN stack + CLS-query MHA).

Self-contained: builds the Bass/Tile program, shards inputs across 8
NeuronCores (data-parallel over destination-node ranges; graphs 4c..4c+3
on core c), runs via run_bass_kernel_spmd, and reassembles the full
[32, 128] output.

Design (vs. the 590us baseline):
  - fp8(e4m3) tables wherever the DMA/PE cost rewards it: gather tables
    (h1, h2), ct count-matrix, T1, W2/W3, selection matrices.  Gather
    rows are 256B (the dma_gather minimum granularity).
  - Edge slots are compacted per 128-dst chunk (128-aligned), and
    dma_gather calls are 1024-index windows decoupled from chunk
    boundaries (the gather ucode caps at 1024 idxs/call).
  - DoubleRow fp8 matmuls (0.5 cyc/row) for the ct and selection
    segment-sum groups and the node matmuls.
  - Selection matrices are built once on DVE (overlapped with layer 1)
    and reused by layers 2 and 3 (identical slotting).
  - The "h + msg" own-row add runs on DVE against the chunk PSUM from
    node-major own tables; biases fold into the own table (L1) or the
    feature-major gelu (L2/L3).
  - The inter-layer AllGather is split into 8 per-segment pieces that
    overlap the producing layer's compute.
  - Attention: K-projection folded into the query on the host
    (scoresT = hT3 @ (Wk qblk); the per-head key-bias constant cancels
    in softmax), scores/exp/V-projection computed node-major per
    segment interleaved with layer 3; no transposes in the tail.
"""

import sys

if "/opt/trn_rl_repo" not in sys.path:
    sys.path.insert(0, "/opt/trn_rl_repo")

import numpy as np
import ml_dtypes

# ---- problem constants (hardcoded per spec) ----
B, N, E, D = 32, 1024, 524288, 128
H, HD = 4, 32
F2 = 2 * D                     # 256
NCORES = 8
NPC = B * N // NCORES          # 4096 nodes per core
GPC = B // NCORES              # 4 graphs per core
CHW = 128                      # dst-chunk width (nodes)
NCH = NPC // CHW               # 32 chunks per core
SEG = 4                        # chunks per segment
NSEG = NCH // SEG              # 8 segments per core
SEGN = SEG * CHW               # 512 nodes per segment
BF16 = ml_dtypes.bfloat16
FP8 = ml_dtypes.float8_e4m3

_prog_cache = {}


def _wrap16(arr):
    """slot i -> [i % 16, i // 16], replicated into partitions 16..31.

    CoreSim's gather ucode reads partitions 0..15; the deployed HW ucode
    reads 16..31 -- fill both so either path sees the indices.
    """
    n = arr.shape[0]
    out = np.zeros((128, n // 16), np.int16)
    w = arr.reshape(n // 16, 16).T.astype(np.int16)
    out[0:16] = w
    out[16:32] = w
    return out


def _host_prep(inputs):
    node_ids = np.asarray(inputs["node_ids"]).astype(np.int64)
    src = np.asarray(inputs["src"]).astype(np.int64)
    dst = np.asarray(inputs["dst"]).astype(np.int64)
    pad_mask = np.asarray(inputs["pad_mask"])
    ew = np.asarray(inputs["edge_weight"]).astype(np.float64)
    embed = np.asarray(inputs["embed_table"]).astype(np.float64)
    W1 = np.asarray(inputs["W1"]).astype(np.float64)
    b1 = np.asarray(inputs["b1"]).astype(np.float32)
    W2 = np.asarray(inputs["W2"]).astype(np.float32)
    b2 = np.asarray(inputs["b2"]).astype(np.float32)
    W3 = np.asarray(inputs["W3"]).astype(np.float32)
    b3 = np.asarray(inputs["b3"]).astype(np.float32)
    ipw = np.asarray(inputs["in_proj_w"]).astype(np.float64)
    ipb = np.asarray(inputs["in_proj_b"]).astype(np.float64)
    ow = np.asarray(inputs["out_w"]).astype(np.float32)
    ob = np.asarray(inputs["out_b"]).astype(np.float32)
    cls = np.asarray(inputs["cls_embedding"]).astype(np.float64).reshape(D)
    ln_g = np.asarray(inputs["ln_g"]).astype(np.float32)
    ln_b = np.asarray(inputs["ln_b"]).astype(np.float32)

    assert not pad_mask.any(), "kernel compiled for all-False pad_mask"

    # ---- shared (replicated) constants ----
    T1 = (embed @ W1).astype(np.float32)                 # [1024, 256]
    t1p = T1.reshape(N // 128, 128, F2).transpose(1, 0, 2).astype(FP8)

    Wq, Wk, Wv = ipw[:, :D], ipw[:, D:2 * D], ipw[:, 2 * D:]
    bq, bk_, bv_ = ipb[:D], ipb[D:2 * D], ipb[2 * D:]
    q_cls = (cls @ Wq + bq) / np.sqrt(HD)                # [128]
    qblk = np.zeros((D, H))
    for h in range(H):
        qblk[h * HD:(h + 1) * HD, h] = q_cls[h * HD:(h + 1) * HD]
    qkf = (Wk @ qblk).astype(np.float32)                 # [128, 4]
    bkq = np.array([bk_ @ qblk[:, h] for h in range(H)])
    k_cls = cls @ Wk + bk_
    s_cls = np.array([q_cls[h * HD:(h + 1) * HD] @ k_cls[h * HD:(h + 1) * HD]
                      for h in range(H)])
    e_cls = np.exp(s_cls - bkq)                          # device scores omit bkq
    v_cls_nb = cls @ Wv                                  # bias added post-softmax
    vc4 = np.zeros((4, 128), np.float32)
    for h in range(H):
        vc4[h, h * HD:(h + 1) * HD] = v_cls_nb[h * HD:(h + 1) * HD]
    e4 = np.diag(e_cls).astype(np.float32)
    msel = np.zeros((128, 4), np.float32)
    r4 = np.zeros((4, 128), np.float32)
    for h in range(H):
        msel[h * HD:(h + 1) * HD, h] = 1.0
        r4[h, h * HD:(h + 1) * HD] = 1.0
    hsel = np.zeros((32, 4), np.float32)
    for j in range(32):
        hsel[j, j % 4] = 1.0

    w2dr = W2.reshape(2, 128, 2, 128).transpose(1, 0, 2, 3).astype(FP8)
    w3dr = W3.reshape(2, 128, 128).transpose(1, 0, 2).astype(FP8)

    # ---- edge slotting (core-uniform: program is SPMD) ----
    ew32 = ew.astype(np.float32)
    order_all = np.argsort(dst, kind='stable')
    dst_sorted = dst[order_all]
    chunk_starts = np.searchsorted(dst_sorted, np.arange(0, B * N + 1, CHW))
    cnt_all = np.diff(chunk_starts).reshape(NCORES, NCH)      # [core, chunk]
    nb_uni = np.maximum(1, -(-cnt_all.max(0) // 128)).astype(np.int64)  # [32]
    B0 = np.concatenate([[0], np.cumsum(nb_uni)]).astype(np.int64)      # [33]
    NBT = int(B0[-1])
    segslots = [int(128 * (B0[SEG * (s + 1)] - B0[SEG * s]))
                for s in range(NSEG)]
    segbase = np.concatenate([[0], np.cumsum(segslots)]).astype(np.int64)
    CAPT = int(segbase[-1])
    SEGBMAX = max(sl // 128 for sl in segslots)

    # agout row permutation: node n -> row (seg<<12 | core<<9 | offset)
    nvec = np.arange(B * N, dtype=np.int64)
    agrow = ((nvec & 4095) >> 9 << 12) | (nvec >> 12 << 9) | (nvec & 511)

    P = dict(nb=tuple(int(x) for x in nb_uni), NBT=NBT, CAPT=CAPT,
             segslots=tuple(segslots), SEGBMAX=SEGBMAX)

    shared = {
        "t1p": t1p,
        "w2dr": w2dr.reshape(128, 2 * 2 * 128),
        "w3dr": w3dr.reshape(128, 2 * 128),
        "b2c": b2.reshape(2, 128).T.copy(),
        "b3c": b3.reshape(128, 1).copy(),
        "wvt": Wv.astype(BF16),
        "qkf": qkf.astype(BF16),
        "i128": np.eye(128, dtype=np.float32).astype(BF16),
        "i128f": np.eye(128, dtype=np.float32),
        "ones128": np.ones((128, 1), np.float32).astype(BF16),
        "hsel": hsel,
        "r4": r4,
        "msel": msel,
        "vc4": vc4.astype(BF16),
        "e4": e4.astype(BF16),
        "ecls": e_cls.astype(np.float32).reshape(4, 1),
        "bvt": bv_.astype(np.float32).reshape(128, 1),
        "ynb": (cls + ob).astype(np.float32).reshape(128, 1),
        "wo": ow.astype(BF16),
        "lng": np.tile(ln_g, (4, 1)),
        "lnb": np.tile(ln_b, (4, 1)),
        "eps": np.full((4, 1), 1e-5, np.float32),
        "cnts": (np.arange(9, dtype=np.int32) * 128).reshape(1, 9),
    }

    T1b = T1 + b1[None, :]                                # b1 folded into own rows
    in_maps = []
    for c in range(NCORES):
        g_idx = np.zeros(CAPT, np.int64)
        selh = np.zeros((NBT * 128, CHW), np.float32)
        for k in range(NCH):
            gk = c * NCH + k
            a, bnd = chunk_starts[gk], chunk_starts[gk + 1]
            cnt = bnd - a
            assert cnt <= 128 * nb_uni[k]
            e = order_all[a:bnd]
            base = 128 * B0[k]
            g_idx[base:base + cnt] = agrow[src[e]]
            dl = (dst[e] - (c * NPC + k * CHW)).astype(np.int64)
            selh[np.arange(base, base + cnt), dl] = ew32[e]
        eidx = order_all[chunk_starts[c * NCH]:chunk_starts[(c + 1) * NCH]]
        ids_e = node_ids[src[eidx]]
        dl_e = dst[eidx] - c * NPC
        Cf = np.bincount(dl_e * N + ids_e, weights=ew[eidx],
                         minlength=NPC * N).reshape(NPC, N).astype(np.float32)
        # ct[p, k, pr, j, d] = Cf[k*128+d, (2pr+j)*128+p]
        ct = Cf.reshape(NCH, CHW, 8, 128).transpose(3, 0, 2, 1).astype(FP8)
        nids_own = node_ids[c * NPC:(c + 1) * NPC]
        # h0own node-major [128, 32, 256], bias b1 folded in
        h0own = T1b[nids_own].reshape(NCH, CHW, F2).transpose(1, 0, 2).astype(FP8)
        m = dict(shared)
        m.update({
            "ct": np.ascontiguousarray(ct).reshape(128, NCH * 8 * CHW),
            "h0own": np.ascontiguousarray(h0own).reshape(128, NCH * F2),
            "idx12": _wrap16(g_idx),
            "selin": np.ascontiguousarray(
                selh.reshape(NBT, 128, CHW).transpose(1, 0, 2)
            ).astype(FP8).reshape(128, NBT * CHW),
        })
        in_maps.append(m)
    return in_maps, P


def _build_program(variant, P):
    key = (variant, P['nb'], P['segslots'])
    if key in _prog_cache:
        return _prog_cache[key]
    import concourse.bacc as bacc
    import concourse.tile as tile
    import concourse.mybir as mybir

    dt = mybir.dt
    AF = mybir.ActivationFunctionType
    OP = mybir.AluOpType
    DR = mybir.MatmulPerfMode.DoubleRow

    nb = P['nb']
    NBT = P['NBT']
    CAPT = P['CAPT']
    segslots = P['segslots']
    SEGBMAX = P['SEGBMAX']
    B0 = [0]
    for x in nb:
        B0.append(B0[-1] + x)
    segbase = [0]
    for sl in segslots:
        segbase.append(segbase[-1] + sl)

    nc = bacc.Bacc("TRN2", target_bir_lowering=False, debug=False,
                   num_devices=(1 if variant == "sim1" else NCORES))

    def din(name, shape, dtype):
        return nc.dram_tensor(name, shape, dtype, kind="ExternalInput")

    t1p = din("t1p", [128, 8, F2], dt.float8e4)
    ct = din("ct", [128, NCH * 8 * CHW], dt.float8e4)
    h0own = din("h0own", [128, NCH * F2], dt.float8e4)
    idx12 = din("idx12", [128, CAPT // 16], dt.int16)
    cnts = din("cnts", [1, 9], dt.int32)
    selin = din("selin", [128, NBT * CHW], dt.float8e4)
    w2dr = din("w2dr", [128, 512], dt.float8e4)
    w3dr = din("w3dr", [128, 256], dt.float8e4)
    b2c = din("b2c", [128, 2], dt.float32)
    b3c = din("b3c", [128, 1], dt.float32)
    wvt = din("wvt", [128, 128], dt.bfloat16)
    qkf = din("qkf", [128, 4], dt.bfloat16)
    i128 = din("i128", [128, 128], dt.bfloat16)
    i128f = din("i128f", [128, 128], dt.float32)
    ones128 = din("ones128", [128, 1], dt.bfloat16)
    hsel = din("hsel", [32, 4], dt.float32)
    r4 = din("r4", [4, 128], dt.float32)
    msel = din("msel", [128, 4], dt.float32)
    vc4 = din("vc4", [4, 128], dt.bfloat16)
    e4 = din("e4", [4, 4], dt.bfloat16)
    ecls = din("ecls", [4, 1], dt.float32)
    bvt = din("bvt", [128, 1], dt.float32)
    ynb = din("ynb", [128, 1], dt.float32)
    wo = din("wo", [128, 128], dt.bfloat16)
    lng = din("lng", [4, 128], dt.float32)
    lnb = din("lnb", [4, 128], dt.float32)
    eps = din("eps", [4, 1], dt.float32)
    y_out = nc.dram_tensor("y", [GPC, D], dt.float32, kind="ExternalOutput")

    with tile.TileContext(nc) as tc:
        from concourse.library_config import mlp
        nc.gpsimd.load_library(mlp)
        with tc.tile_pool(name="const", bufs=1) as cp, \
             tc.tile_pool(name="res", bufs=1) as rp, \
             tc.tile_pool(name="work", bufs=1) as wp, \
             tc.tile_pool(name="ps", bufs=1, space="PSUM") as pp, \
             tc.tile_pool(name="dram", bufs=2, space="DRAM") as dram:

            def cload(ap, shape, dtype):
                t = cp.tile(shape, dtype, name=f"c_{ap.name}")
                nc.sync.dma_start(out=t[:], in_=ap[:])
                return t

            t1p_t = cload(t1p, [128, 8, F2], dt.float8e4)
            h0own_t = cload(h0own, [128, NCH * F2], dt.float8e4)
            h0own_v = h0own_t[:].rearrange("p (k f) -> p k f", f=F2)
            idx_t = cload(idx12, [128, CAPT // 16], dt.int16)
            cnts_t = cload(cnts, [1, 9], dt.int32)
            w2_t = cload(w2dr, [128, 512], dt.float8e4)
            w2_v = w2_t[:].rearrange("p (t jo o) -> p t jo o", t=2, jo=2)
            w3_t = cload(w3dr, [128, 256], dt.float8e4)
            w3_v = w3_t[:].rearrange("p (t o) -> p t o", t=2)
            b2c_t = cload(b2c, [128, 2], dt.float32)
            b3c_t = cload(b3c, [128, 1], dt.float32)
            wvt_t = cload(wvt, [128, 128], dt.bfloat16)
            qkf_t = cload(qkf, [128, 4], dt.bfloat16)
            i128_t = cload(i128, [128, 128], dt.bfloat16)
            i128f_t = cload(i128f, [128, 128], dt.float32)
            ones_t = cload(ones128, [128, 1], dt.bfloat16)
            hsel_t = cload(hsel, [32, 4], dt.float32)
            r4_t = cload(r4, [4, 128], dt.float32)
            msel_t = cload(msel, [128, 4], dt.float32)
            vc4_t = cload(vc4, [4, 128], dt.bfloat16)
            e4_t = cload(e4, [4, 4], dt.bfloat16)
            ecls_t = cload(ecls, [4, 1], dt.float32)
            bvt_t = cload(bvt, [128, 1], dt.float32)
            ynb_t = cload(ynb, [128, 1], dt.float32)
            wo_t = cload(wo, [128, 128], dt.bfloat16)
            lng_t = cload(lng, [4, 128], dt.float32)
            lnb_t = cload(lnb, [4, 128], dt.float32)
            eps_t = cload(eps, [4, 1], dt.float32)

            sel_t = rp.tile([128, NBT, CHW], dt.float8e4, name="sel")
            hon1 = rp.tile([128, NCH, F2], dt.float8e4, name="hon1")
            hon2 = rp.tile([128, NCH, F2], dt.float8e4, name="hon2")
            rhsT = rp.tile([128, 2, NPC], dt.float8e4, name="rhsT")
            hT3 = rp.tile([128, NPC], dt.bfloat16, name="hT3")
            vnm = rp.tile([128, NPC // 128, 128], dt.bfloat16, name="vnm")
            esc = rp.tile([128, 128], dt.bfloat16, name="esc")
            ctx_all = rp.tile([128, 4], dt.bfloat16, name="ctx_all")
            ctbufs = [wp.tile([128, SEG * 8 * CHW], dt.float8e4, tag=f"ctb{i}",
                              name=f"ctb{i}") for i in range(2)]
            gbufs = [wp.tile([128, SEGBMAX, F2], dt.float8e4, tag=f"gb{i}",
                             name=f"gbuf{i}") for i in range(2)]

            agin1 = dram.tile([NPC, F2], dt.float8e4, tag="agin1")
            agout1 = dram.tile([B * N, F2], dt.float8e4, tag="agout1")
            agin2 = dram.tile([NPC, F2], dt.float8e4, tag="agin2")
            agout2 = dram.tile([B * N, F2], dt.float8e4, tag="agout2")

            hon1_v = hon1[:]
            hon2_v = hon2[:]
            state = {"hon_cur": hon1_v}

            def ag_piece(s, agin, agout):
                # send this core's segment-s rows, receive everyone's
                agin_v = agin.rearrange("(g tt p) f -> p g tt f", tt=SEG, p=CHW)
                nc.sync.dma_start(out=agin_v[:, s, :, :],
                                  in_=state["hon_cur"][:, SEG * s:SEG * (s + 1), :])
                if variant == "sim1":
                    for cc in range(NCORES):
                        nc.sync.dma_start(
                            out=agout[s * NPC + cc * SEGN:
                                      s * NPC + (cc + 1) * SEGN, :],
                            in_=agin[s * SEGN:(s + 1) * SEGN, :])
                else:
                    nc.gpsimd.collective_compute(
                        "AllGather", mybir.AluOpType.bypass,
                        replica_groups=[list(range(NCORES))],
                        ins=[agin[s * SEGN:(s + 1) * SEGN, :].opt()],
                        outs=[agout[s * NPC:(s + 1) * NPC, :].opt()])

            # ---------------- layer 1 (ct x T1) ----------------
            nc.sync.dma_start(out=ctbufs[0][:], in_=ct[:, 0:SEG * 8 * CHW])
            for s in range(NSEG):
                if s + 1 < NSEG:
                    nc.sync.dma_start(
                        out=ctbufs[(s + 1) % 2][:],
                        in_=ct[:, (s + 1) * SEG * 8 * CHW:(s + 2) * SEG * 8 * CHW])
                ctb_v = ctbufs[s % 2][:].rearrange(
                    "p (kk pr j d) -> p kk pr j d", pr=4, j=2, d=CHW)
                for kk in range(SEG):
                    k = SEG * s + kk
                    ps = pp.tile([CHW, F2], dt.float32, tag="big", bufs=2)
                    for pr in range(4):
                        nc.tensor.matmul(
                            out=ps[:], lhsT=ctb_v[:, kk, pr, :, :],
                            rhs=t1p_t[:, 2 * pr:2 * pr + 2, :],
                            perf_mode=DR, start=(pr == 0), stop=(pr == 3),
                            skip_group_check=True)
                    msb = wp.tile([CHW, F2], dt.bfloat16, tag="msb", bufs=3)
                    nc.vector.tensor_tensor(out=msb[:], in0=ps[:],
                                            in1=h0own_v[:, k, :], op=OP.add)
                    nc.scalar.activation(hon1_v[:, k, :], msb[:], AF.Gelu)
                # stream this segment's prebuilt selection matrices
                nbs = segslots[s] // 128
                b0 = B0[SEG * s]
                nc.sync.dma_start(
                    out=sel_t[:, b0:b0 + nbs, :],
                    in_=selin[:, b0 * CHW:(b0 + nbs) * CHW].rearrange(
                        "p (b d) -> p b d", d=CHW))
                ag_piece(s, agin1, agout1)

            # ---------------- layers 2 and 3 ----------------
            cregs = {}
            for layer in (2, 3):
                table = agout1 if layer == 2 else agout2
                own = hon1_v if layer == 2 else hon2_v
                for s in range(NSEG):
                    gb = gbufs[s % 2]
                    ss_ = segslots[s]
                    ci = 0
                    while ci * 1024 < ss_:
                        w0 = ci * 1024
                        ni = min(1024, ss_ - w0)
                        nbw = ni // 128
                        if nbw not in cregs:
                            cregs[nbw] = nc.gpsimd.value_load(
                                cnts_t[0:1, nbw:nbw + 1])
                        i0 = (segbase[s] + w0) // 16
                        nc.gpsimd.dma_gather(
                            gb[:, w0 // 128:w0 // 128 + nbw, :], table[:],
                            idx_t[:, i0:i0 + ni // 16],
                            ni, cregs[nbw], F2)
                        ci += 1
                    for kk in range(SEG):
                        k = SEG * s + kk
                        nbk = nb[k]
                        b0 = B0[k]
                        g0 = B0[k] - B0[SEG * s]
                        ps = pp.tile([CHW, F2], dt.float32, tag="big", bufs=2)
                        j = 0
                        while j < nbk:
                            if j + 1 < nbk:
                                nc.tensor.matmul(
                                    out=ps[:],
                                    lhsT=sel_t[:, b0 + j:b0 + j + 2, :],
                                    rhs=gb[:, g0 + j:g0 + j + 2, :],
                                    perf_mode=DR, start=(j == 0),
                                    stop=(j + 2 >= nbk), skip_group_check=True)
                                j += 2
                            else:
                                nc.tensor.matmul(
                                    out=ps[:], lhsT=sel_t[:, b0 + j, :],
                                    rhs=gb[:, g0 + j, :],
                                    start=(j == 0), stop=True,
                                    skip_group_check=True)
                                j += 1
                        msb = wp.tile([CHW, F2], dt.bfloat16, tag="msb", bufs=3)
                        nc.vector.tensor_tensor(out=msb[:], in0=ps[:],
                                                in1=own[:, k, :], op=OP.add)
                        for jj in range(2):
                            tp = pp.tile([128, 128], dt.bfloat16, tag="tp", bufs=2)
                            nc.tensor.transpose(
                                tp[:], msb[:, jj * 128:(jj + 1) * 128], i128_t[:])
                            nc.vector.tensor_copy(
                                out=rhsT[:, jj, k * CHW:(k + 1) * CHW], in_=tp[:])
                    # node matmul + gelu for this segment
                    if layer == 2:
                        for jo in range(2):
                            psz = pp.tile([128, SEGN], dt.float32, tag="big",
                                          bufs=2)
                            nc.tensor.matmul(
                                out=psz[:], lhsT=w2_v[:, :, jo, :],
                                rhs=rhsT[:, :, s * SEGN:(s + 1) * SEGN],
                                perf_mode=DR, start=True, stop=True,
                                skip_group_check=True)
                            hf = wp.tile([128, SEGN], dt.bfloat16, tag="hf",
                                         bufs=2)
                            nc.scalar.activation(hf[:], psz[:], AF.Gelu,
                                                 bias=b2c_t[:, jo:jo + 1])
                            for tt in range(SEG):
                                tp2 = pp.tile([128, 128], dt.bfloat16,
                                              tag="tp", bufs=2)
                                nc.tensor.transpose(
                                    tp2[:], hf[:, tt * 128:(tt + 1) * 128],
                                    i128_t[:])
                                nc.vector.tensor_copy(
                                    out=hon2_v[:, SEG * s + tt,
                                               jo * 128:(jo + 1) * 128],
                                    in_=tp2[:])
                        state["hon_cur"] = hon2_v
                        ag_piece(s, agin2, agout2)
                    else:
                        psz = pp.tile([128, SEGN], dt.float32, tag="big", bufs=2)
                        nc.tensor.matmul(
                            out=psz[:], lhsT=w3_v[:, :, :],
                            rhs=rhsT[:, :, s * SEGN:(s + 1) * SEGN],
                            perf_mode=DR, start=True, stop=True,
                            skip_group_check=True)
                        nc.scalar.activation(hT3[:, s * SEGN:(s + 1) * SEGN],
                                             psz[:], AF.Gelu, bias=b3c_t[:, 0:1])
                        # attention tiles for this segment
                        psc = pp.tile([128, 16], dt.float32, tag="psc", bufs=1)
                        for tt in range(4):
                            t = 4 * s + tt
                            nc.tensor.matmul(
                                out=psc[:, tt * 4:(tt + 1) * 4],
                                lhsT=hT3[:, t * 128:(t + 1) * 128], rhs=qkf_t[:],
                                start=True, stop=True)
                            psv = pp.tile([128, 128], dt.float32, tag="att",
                                          bufs=2)
                            nc.tensor.matmul(
                                out=psv[:], lhsT=hT3[:, t * 128:(t + 1) * 128],
                                rhs=wvt_t[:], start=True, stop=True)
                            nc.vector.tensor_copy(out=vnm[:, t, :], in_=psv[:])
                        nc.scalar.activation(esc[:, s * 16:(s + 1) * 16],
                                             psc[:], AF.Exp)
                        if s % 2 == 1:
                            g = s // 2
                            psE = pp.tile([32, 1], dt.float32, tag="att", bufs=2)
                            nc.tensor.matmul(out=psE[:],
                                             lhsT=esc[:, g * 32:(g + 1) * 32],
                                             rhs=ones_t[:], start=True, stop=True)
                            s32 = wp.tile([32, 1], dt.float32, tag="s32", bufs=2)
                            nc.vector.tensor_copy(out=s32[:], in_=psE[:])
                            ps4 = pp.tile([4, 1], dt.float32, tag="att", bufs=2)
                            nc.tensor.matmul(out=ps4[:], lhsT=hsel_t[:],
                                             rhs=s32[:], start=True, stop=True)
                            sums4 = wp.tile([4, 1], dt.float32, tag="sums4",
                                            bufs=2)
                            nc.vector.tensor_tensor(out=sums4[:], in0=ps4[:],
                                                    in1=ecls_t[:], op=OP.add)
                            rr4 = wp.tile([4, 1], dt.float32, tag="rr4", bufs=2)
                            nc.vector.reciprocal(rr4[:], sums4[:])
                            psr = pp.tile([128, 1], dt.float32, tag="att", bufs=2)
                            nc.tensor.matmul(out=psr[:], lhsT=r4_t[:],
                                             rhs=rr4[:], start=True, stop=True)
                            rbc = wp.tile([128, 1], dt.float32, tag="rbc", bufs=2)
                            nc.vector.tensor_copy(out=rbc[:], in_=psr[:])
                            psg = pp.tile([128, 4], dt.float32, tag="att", bufs=2)
                            for t in range(8):
                                nc.tensor.matmul(
                                    out=psg[:], lhsT=vnm[:, 8 * g + t, :],
                                    rhs=esc[:, (8 * g + t) * 4:
                                            (8 * g + t + 1) * 4],
                                    start=(t == 0), stop=False,
                                    skip_group_check=True)
                            nc.tensor.matmul(out=psg[:], lhsT=vc4_t[:],
                                             rhs=e4_t[:], start=False, stop=True,
                                             skip_group_check=True)
                            tmp4 = wp.tile([128, 4], dt.float32, tag="tmp4",
                                           bufs=2)
                            nc.vector.tensor_tensor(out=tmp4[:], in0=psg[:],
                                                    in1=msel_t[:], op=OP.mult)
                            ctxv = wp.tile([128, 1], dt.float32, tag="ctxv",
                                           bufs=2)
                            nc.vector.reduce_sum(out=ctxv[:], in_=tmp4[:],
                                                 axis=mybir.AxisListType.X)
                            nc.vector.tensor_scalar(
                                out=ctx_all[:, g:g + 1], in0=ctxv[:],
                                scalar1=rbc[:], scalar2=bvt_t[:],
                                op0=OP.mult, op1=OP.add)

            # ---------------- output projection + LayerNorm ----------------
            psao = pp.tile([128, 4], dt.float32, tag="att", bufs=2)
            nc.tensor.matmul(out=psao[:], lhsT=wo_t[:], rhs=ctx_all[:],
                             start=True, stop=True)
            ysb = wp.tile([128, 4], dt.float32, tag="ysb")
            nc.vector.tensor_scalar(out=ysb[:], in0=psao[:],
                                    scalar1=ynb_t[:], scalar2=None, op0=OP.add)
            psy = pp.tile([4, 128], dt.float32, tag="att", bufs=2)
            nc.tensor.matmul(out=psy[:], lhsT=ysb[:], rhs=i128f_t[:],
                             is_transpose=True)
            yt = wp.tile([4, 128], dt.float32, tag="yt")
            nc.vector.tensor_copy(out=yt[:], in_=psy[:])
            mn = wp.tile([4, 1], dt.float32, tag="mn")
            nc.vector.reduce_sum(out=mn[:], in_=yt[:], axis=mybir.AxisListType.X)
            nc.vector.tensor_scalar(out=mn[:], in0=mn[:], scalar1=1.0 / D,
                                    scalar2=None, op0=OP.mult)
            xc = wp.tile([4, 128], dt.float32, tag="xc")
            nc.vector.tensor_scalar(out=xc[:], in0=yt[:], scalar1=mn[:],
                                    scalar2=None, op0=OP.subtract)
            sq = wp.tile([4, 128], dt.float32, tag="sq")
            ss = wp.tile([4, 1], dt.float32, tag="ss")
            nc.scalar.activation(sq[:], xc[:], AF.Square, accum_out=ss[:])
            sd = wp.tile([4, 1], dt.float32, tag="sd")
            nc.scalar.activation(sd[:], ss[:], AF.Sqrt, bias=eps_t[:],
                                 scale=1.0 / D)
            rr = wp.tile([4, 1], dt.float32, tag="rr")
            nc.vector.reciprocal(rr[:], sd[:])
            yn = wp.tile([4, 128], dt.float32, tag="yn")
            nc.vector.tensor_scalar(out=yn[:], in0=xc[:], scalar1=rr[:],
                                    scalar2=None, op0=OP.mult)
            nc.vector.tensor_tensor(out=yn[:], in0=yn[:], in1=lng_t[:],
                                    op=OP.mult)
            nc.vector.tensor_tensor(out=yn[:], in0=yn[:], in1=lnb_t[:],
                                    op=OP.add)
            nc.sync.dma_start(out=y_out[:], in_=yn[:])

    nc.compile()
    _prog_cache[key] = nc
    return nc


def kernel(**inputs):
    from concourse.bass_utils import run_bass_kernel_spmd
    in_maps, P = _host_prep(inputs)
    nc = _build_program("hw", P)
    res = run_bass_kernel_spmd(nc, in_maps, core_ids=list(range(NCORES)))
    y = np.concatenate([res.results[c]["y"] for c in range(NCORES)], axis=0)
    return np.ascontiguousarray(y.astype(np.float32))


# revision 6
# speedup vs baseline: 1.7219x; 1.0326x over previous
"""Trainium2 Bass kernel for nn_GTShapelet (GIN stack + CLS-query MHA).

Self-contained: builds the Bass/Tile program, shards inputs across 8
NeuronCores (data-parallel over destination-node ranges; graphs 4c..4c+3
on core c), runs via run_bass_kernel_spmd, and reassembles the full
[32, 128] output.

Design (vs. the 590us baseline):
  - fp8(e4m3) tables wherever the DMA/PE cost rewards it: gather tables
    (h1, h2), ct count-matrix, T1, W2/W3, selection matrices.  Gather
    rows are 256B (the dma_gather minimum granularity).
  - Edge slots are compacted per 128-dst chunk (128-aligned), and
    dma_gather calls are 1024-index windows decoupled from chunk
    boundaries (the gather ucode caps at 1024 idxs/call).
  - DoubleRow fp8 matmuls (0.5 cyc/row) for the ct and selection
    segment-sum groups and the node matmuls.
  - Selection matrices are built once on DVE (overlapped with layer 1)
    and reused by layers 2 and 3 (identical slotting).
  - The "h + msg" own-row add runs on DVE against the chunk PSUM from
    node-major own tables; biases fold into the own table (L1) or the
    feature-major gelu (L2/L3).
  - The inter-layer AllGather is split into 8 per-segment pieces that
    overlap the producing layer's compute.
  - Attention: K-projection folded into the query on the host
    (scoresT = hT3 @ (Wk qblk); the per-head key-bias constant cancels
    in softmax), scores/exp/V-projection computed node-major per
    segment interleaved with layer 3; no transposes in the tail.
"""

import sys

if "/opt/trn_rl_repo" not in sys.path:
    sys.path.insert(0, "/opt/trn_rl_repo")

import numpy as np
import ml_dtypes

# ---- problem constants (hardcoded per spec) ----
B, N, E, D = 32, 1024, 524288, 128
H, HD = 4, 32
F2 = 2 * D                     # 256
NCORES = 8
NPC = B * N // NCORES          # 4096 nodes per core
GPC = B // NCORES              # 4 graphs per core
CHW = 128                      # dst-chunk width (nodes)
NCH = NPC // CHW               # 32 chunks per core
SEG = 4                        # chunks per segment
NSEG = NCH // SEG              # 8 segments per core
SEGN = SEG * CHW               # 512 nodes per segment
BF16 = ml_dtypes.bfloat16
FP8 = ml_dtypes.float8_e4m3

_prog_cache = {}


def _wrap16(arr):
    """slot i -> [i % 16, i // 16], replicated into partitions 16..31.

    CoreSim's gather ucode reads partitions 0..15; the deployed HW ucode
    reads 16..31 -- fill both so either path sees the indices.
    """
    n = arr.shape[0]
    out = np.zeros((128, n // 16), np.int16)
    w = arr.reshape(n // 16, 16).T.astype(np.int16)
    out[0:16] = w
    out[16:32] = w
    return out


def _host_prep(inputs):
    node_ids = np.asarray(inputs["node_ids"]).astype(np.int64)
    src = np.asarray(inputs["src"]).astype(np.int64)
    dst = np.asarray(inputs["dst"]).astype(np.int64)
    pad_mask = np.asarray(inputs["pad_mask"])
    ew = np.asarray(inputs["edge_weight"]).astype(np.float64)
    embed = np.asarray(inputs["embed_table"]).astype(np.float64)
    W1 = np.asarray(inputs["W1"]).astype(np.float64)
    b1 = np.asarray(inputs["b1"]).astype(np.float32)
    W2 = np.asarray(inputs["W2"]).astype(np.float32)
    b2 = np.asarray(inputs["b2"]).astype(np.float32)
    W3 = np.asarray(inputs["W3"]).astype(np.float32)
    b3 = np.asarray(inputs["b3"]).astype(np.float32)
    ipw = np.asarray(inputs["in_proj_w"]).astype(np.float64)
    ipb = np.asarray(inputs["in_proj_b"]).astype(np.float64)
    ow = np.asarray(inputs["out_w"]).astype(np.float32)
    ob = np.asarray(inputs["out_b"]).astype(np.float32)
    cls = np.asarray(inputs["cls_embedding"]).astype(np.float64).reshape(D)
    ln_g = np.asarray(inputs["ln_g"]).astype(np.float32)
    ln_b = np.asarray(inputs["ln_b"]).astype(np.float32)

    assert not pad_mask.any(), "kernel compiled for all-False pad_mask"

    # ---- shared (replicated) constants ----
    T1 = (embed @ W1).astype(np.float32)                 # [1024, 256]
    t1p = T1.reshape(N // 128, 128, F2).transpose(1, 0, 2).astype(FP8)

    Wq, Wk, Wv = ipw[:, :D], ipw[:, D:2 * D], ipw[:, 2 * D:]
    bq, bk_, bv_ = ipb[:D], ipb[D:2 * D], ipb[2 * D:]
    q_cls = (cls @ Wq + bq) / np.sqrt(HD)                # [128]
    qblk = np.zeros((D, H))
    for h in range(H):
        qblk[h * HD:(h + 1) * HD, h] = q_cls[h * HD:(h + 1) * HD]
    qkf = (Wk @ qblk).astype(np.float32)                 # [128, 4]
    bkq = np.array([bk_ @ qblk[:, h] for h in range(H)])
    k_cls = cls @ Wk + bk_
    s_cls = np.array([q_cls[h * HD:(h + 1) * HD] @ k_cls[h * HD:(h + 1) * HD]
                      for h in range(H)])
    e_cls = np.exp(s_cls - bkq)                          # device scores omit bkq
    v_cls_nb = cls @ Wv                                  # bias added post-softmax
    vc4 = np.zeros((4, 128), np.float32)
    for h in range(H):
        vc4[h, h * HD:(h + 1) * HD] = v_cls_nb[h * HD:(h + 1) * HD]
    e4 = np.diag(e_cls).astype(np.float32)
    msel = np.zeros((128, 4), np.float32)
    r4 = np.zeros((4, 128), np.float32)
    for h in range(H):
        msel[h * HD:(h + 1) * HD, h] = 1.0
        r4[h, h * HD:(h + 1) * HD] = 1.0
    hsel = np.zeros((32, 4), np.float32)
    for j in range(32):
        hsel[j, j % 4] = 1.0

    w2dr = W2.reshape(2, 128, 2, 128).transpose(1, 0, 2, 3).astype(FP8)
    w3dr = W3.reshape(2, 128, 128).transpose(1, 0, 2).astype(FP8)

    # ---- edge slotting (core-uniform: program is SPMD) ----
    ew32 = ew.astype(np.float32)
    order_all = np.argsort(dst, kind='stable')
    dst_sorted = dst[order_all]
    chunk_starts = np.searchsorted(dst_sorted, np.arange(0, B * N + 1, CHW))
    cnt_all = np.diff(chunk_starts).reshape(NCORES, NCH)      # [core, chunk]
    nb_uni = np.maximum(1, -(-cnt_all.max(0) // 128)).astype(np.int64)  # [32]
    B0 = np.concatenate([[0], np.cumsum(nb_uni)]).astype(np.int64)      # [33]
    NBT = int(B0[-1])
    segslots = [int(128 * (B0[SEG * (s + 1)] - B0[SEG * s]))
                for s in range(NSEG)]
    segbase = np.concatenate([[0], np.cumsum(segslots)]).astype(np.int64)
    CAPT = int(segbase[-1])
    SEGBMAX = max(sl // 128 for sl in segslots)

    # agout row permutation: node n -> row (seg<<12 | core<<9 | offset)
    nvec = np.arange(B * N, dtype=np.int64)
    agrow = ((nvec & 4095) >> 9 << 12) | (nvec >> 12 << 9) | (nvec & 511)

    P = dict(nb=tuple(int(x) for x in nb_uni), NBT=NBT, CAPT=CAPT,
             segslots=tuple(segslots), SEGBMAX=SEGBMAX)

    shared = {
        "t1p": t1p,
        "w2dr": w2dr.reshape(128, 2 * 2 * 128),
        "w3dr": w3dr.reshape(128, 2 * 128),
        "b2c": b2.reshape(2, 128).T.copy(),
        "b3c": b3.reshape(128, 1).copy(),
        "wvt": Wv.astype(BF16),
        "qkf": qkf.astype(BF16),
        "i128": np.eye(128, dtype=np.float32).astype(BF16),
        "i128f": np.eye(128, dtype=np.float32),
        "ones128": np.ones((128, 1), np.float32).astype(BF16),
        "hsel": hsel,
        "r4": r4,
        "msel": msel,
        "vc4": vc4.astype(BF16),
        "e4": e4.astype(BF16),
        "ecls": e_cls.astype(np.float32).reshape(4, 1),
        "bvt": bv_.astype(np.float32).reshape(128, 1),
        "ynb": (cls + ob).astype(np.float32).reshape(128, 1),
        "wo": ow.astype(BF16),
        "lng": np.tile(ln_g, (4, 1)),
        "lnb": np.tile(ln_b, (4, 1)),
        "eps": np.full((4, 1), 1e-5, np.float32),
        "cnts": (np.arange(9, dtype=np.int32) * 128).reshape(1, 9),
    }

    T1b = T1 + b1[None, :]                                # b1 folded into own rows
    in_maps = []
    for c in range(NCORES):
        g_idx = np.zeros(CAPT, np.int64)
        selh = np.zeros((NBT * 128, CHW), np.float32)
        for k in range(NCH):
            gk = c * NCH + k
            a, bnd = chunk_starts[gk], chunk_starts[gk + 1]
            cnt = bnd - a
            assert cnt <= 128 * nb_uni[k]
            e = order_all[a:bnd]
            base = 128 * B0[k]
            g_idx[base:base + cnt] = agrow[src[e]]
            dl = (dst[e] - (c * NPC + k * CHW)).astype(np.int64)
            selh[np.arange(base, base + cnt), dl] = ew32[e]
        eidx = order_all[chunk_starts[c * NCH]:chunk_starts[(c + 1) * NCH]]
        ids_e = node_ids[src[eidx]]
        dl_e = dst[eidx] - c * NPC
        Cf = np.bincount(dl_e * N + ids_e, weights=ew[eidx],
                         minlength=NPC * N).reshape(NPC, N).astype(np.float32)
        # ct[p, k, pr, j, d] = Cf[k*128+d, (2pr+j)*128+p]
        ct = Cf.reshape(NCH, CHW, 8, 128).transpose(3, 0, 2, 1).astype(FP8)
        nids_own = node_ids[c * NPC:(c + 1) * NPC]
        # h0own node-major [128, 32, 256], bias b1 folded in
        h0own = T1b[nids_own].reshape(NCH, CHW, F2).transpose(1, 0, 2).astype(FP8)
        m = dict(shared)
        m.update({
            "ct": np.ascontiguousarray(ct).reshape(128, NCH * 8 * CHW),
            "h0own": np.ascontiguousarray(h0own).reshape(128, NCH * F2),
            "idx12": _wrap16(g_idx),
            "selin": np.ascontiguousarray(
                selh.reshape(NBT, 128, CHW).transpose(1, 0, 2)
            ).astype(FP8).reshape(128, NBT * CHW),
        })
        in_maps.append(m)
    return in_maps, P


def _build_program(variant, P):
    key = (variant, P['nb'], P['segslots'])
    if key in _prog_cache:
        return _prog_cache[key]
    import concourse.bacc as bacc
    import concourse.tile as tile
    import concourse.mybir as mybir

    dt = mybir.dt
    AF = mybir.ActivationFunctionType
    OP = mybir.AluOpType
    DR = mybir.MatmulPerfMode.DoubleRow

    nb = P['nb']
    NBT = P['NBT']
    CAPT = P['CAPT']
    segslots = P['segslots']
    SEGBMAX = P['SEGBMAX']
    B0 = [0]
    for x in nb:
        B0.append(B0[-1] + x)
    segbase = [0]
    for sl in segslots:
        segbase.append(segbase[-1] + sl)

    nc = bacc.Bacc("TRN2", target_bir_lowering=False, debug=False,
                   num_devices=(1 if variant == "sim1" else NCORES))

    def din(name, shape, dtype):
        return nc.dram_tensor(name, shape, dtype, kind="ExternalInput")

    t1p = din("t1p", [128, 8, F2], dt.float8e4)
    ct = din("ct", [128, NCH * 8 * CHW], dt.float8e4)
    h0own = din("h0own", [128, NCH * F2], dt.float8e4)
    idx12 = din("idx12", [128, CAPT // 16], dt.int16)
    cnts = din("cnts", [1, 9], dt.int32)
    selin = din("selin", [128, NBT * CHW], dt.float8e4)
    w2dr = din("w2dr", [128, 512], dt.float8e4)
    w3dr = din("w3dr", [128, 256], dt.float8e4)
    b2c = din("b2c", [128, 2], dt.float32)
    b3c = din("b3c", [128, 1], dt.float32)
    wvt = din("wvt", [128, 128], dt.bfloat16)
    qkf = din("qkf", [128, 4], dt.bfloat16)
    i128 = din("i128", [128, 128], dt.bfloat16)
    i128f = din("i128f", [128, 128], dt.float32)
    ones128 = din("ones128", [128, 1], dt.bfloat16)
    hsel = din("hsel", [32, 4], dt.float32)
    r4 = din("r4", [4, 128], dt.float32)
    msel = din("msel", [128, 4], dt.float32)
    vc4 = din("vc4", [4, 128], dt.bfloat16)
    e4 = din("e4", [4, 4], dt.bfloat16)
    ecls = din("ecls", [4, 1], dt.float32)
    bvt = din("bvt", [128, 1], dt.float32)
    ynb = din("ynb", [128, 1], dt.float32)
    wo = din("wo", [128, 128], dt.bfloat16)
    lng = din("lng", [4, 128], dt.float32)
    lnb = din("lnb", [4, 128], dt.float32)
    eps = din("eps", [4, 1], dt.float32)
    y_out = nc.dram_tensor("y", [GPC, D], dt.float32, kind="ExternalOutput")

    with tile.TileContext(nc) as tc:
        from concourse.library_config import mlp
        nc.gpsimd.load_library(mlp)
        with tc.tile_pool(name="const", bufs=1) as cp, \
             tc.tile_pool(name="res", bufs=1) as rp, \
             tc.tile_pool(name="work", bufs=1) as wp, \
             tc.tile_pool(name="ps", bufs=1, space="PSUM") as pp, \
             tc.tile_pool(name="dram", bufs=2, space="DRAM") as dram:

            def cload(ap, shape, dtype):
                t = cp.tile(shape, dtype, name=f"c_{ap.name}")
                nc.sync.dma_start(out=t[:], in_=ap[:])
                return t

            t1p_t = cload(t1p, [128, 8, F2], dt.float8e4)
            ctbufs = [wp.tile([128, SEG * 8 * CHW], dt.float8e4, tag=f"ctb{i}",
                              name=f"ctb{i}") for i in range(2)]
            nc.sync.dma_start(out=ctbufs[0][:], in_=ct[:, 0:SEG * 8 * CHW])
            h0own_t = cload(h0own, [128, NCH * F2], dt.float8e4)
            h0own_v = h0own_t[:].rearrange("p (k f) -> p k f", f=F2)
            idx_t = cload(idx12, [128, CAPT // 16], dt.int16)
            cnts_t = cload(cnts, [1, 9], dt.int32)
            w2_t = cload(w2dr, [128, 512], dt.float8e4)
            w2_v = w2_t[:].rearrange("p (t jo o) -> p t jo o", t=2, jo=2)
            w3_t = cload(w3dr, [128, 256], dt.float8e4)
            w3_v = w3_t[:].rearrange("p (t o) -> p t o", t=2)
            b2c_t = cload(b2c, [128, 2], dt.float32)
            b3c_t = cload(b3c, [128, 1], dt.float32)
            wvt_t = cload(wvt, [128, 128], dt.bfloat16)
            qkf_t = cload(qkf, [128, 4], dt.bfloat16)
            i128_t = cload(i128, [128, 128], dt.bfloat16)
            i128f_t = cload(i128f, [128, 128], dt.float32)
            ones_t = cload(ones128, [128, 1], dt.bfloat16)
            hsel_t = cload(hsel, [32, 4], dt.float32)
            r4_t = cload(r4, [4, 128], dt.float32)
            msel_t = cload(msel, [128, 4], dt.float32)
            vc4_t = cload(vc4, [4, 128], dt.bfloat16)
            e4_t = cload(e4, [4, 4], dt.bfloat16)
            ecls_t = cload(ecls, [4, 1], dt.float32)
            bvt_t = cload(bvt, [128, 1], dt.float32)
            ynb_t = cload(ynb, [128, 1], dt.float32)
            wo_t = cload(wo, [128, 128], dt.bfloat16)
            lng_t = cload(lng, [4, 128], dt.float32)
            lnb_t = cload(lnb, [4, 128], dt.float32)
            eps_t = cload(eps, [4, 1], dt.float32)

            sel_t = rp.tile([128, NBT, CHW], dt.float8e4, name="sel")
            hon1 = rp.tile([128, NCH, F2], dt.float8e4, name="hon1")
            hon2 = rp.tile([128, NCH, F2], dt.float8e4, name="hon2")
            rhsT = rp.tile([128, 2, NPC], dt.float8e4, name="rhsT")
            hT3 = rp.tile([128, NPC], dt.bfloat16, name="hT3")
            vnm = rp.tile([128, NPC // 128, 128], dt.bfloat16, name="vnm")
            esc = rp.tile([128, 128], dt.bfloat16, name="esc")
            ctx_all = rp.tile([128, 4], dt.bfloat16, name="ctx_all")
            gbufs = [wp.tile([128, SEGBMAX, F2], dt.float8e4, tag=f"gb{i}",
                             name=f"gbuf{i}") for i in range(2)]

            agin1 = dram.tile([NPC, F2], dt.float8e4, tag="agin1")
            agout1 = dram.tile([B * N, F2], dt.float8e4, tag="agout1")
            agin2 = dram.tile([NPC, F2], dt.float8e4, tag="agin2")
            agout2 = dram.tile([B * N, F2], dt.float8e4, tag="agout2")

            hon1_v = hon1[:]
            hon2_v = hon2[:]
            state = {"hon_cur": hon1_v}

            def ag_piece(s, agin, agout):
                # send this core's segment-s rows, receive everyone's
                agin_v = agin.rearrange("(g tt p) f -> p g tt f", tt=SEG, p=CHW)
                nc.sync.dma_start(out=agin_v[:, s, :, :],
                                  in_=state["hon_cur"][:, SEG * s:SEG * (s + 1), :])
                if variant == "sim1":
                    nc.sync.dma_start(
                        out=agout[s * NPC:(s + 1) * NPC, :].rearrange(
                            "(c n) f -> c n f", c=NCORES),
                        in_=agin[s * SEGN:(s + 1) * SEGN, :].unsqueeze(0)
                            .broadcast_to([NCORES, SEGN, F2]))
                else:
                    nc.gpsimd.collective_compute(
                        "AllGather", mybir.AluOpType.bypass,
                        replica_groups=[list(range(NCORES))],
                        ins=[agin[s * SEGN:(s + 1) * SEGN, :].opt()],
                        outs=[agout[s * NPC:(s + 1) * NPC, :].opt()])

            # ---------------- layer 1 (ct x T1) ----------------
            for s in range(NSEG):
                if s + 1 < NSEG:
                    nc.sync.dma_start(
                        out=ctbufs[(s + 1) % 2][:],
                        in_=ct[:, (s + 1) * SEG * 8 * CHW:(s + 2) * SEG * 8 * CHW])
                ctb_v = ctbufs[s % 2][:].rearrange(
                    "p (kk pr j d) -> p kk pr j d", pr=4, j=2, d=CHW)
                for kk in range(SEG):
                    k = SEG * s + kk
                    ps = pp.tile([CHW, F2], dt.float32, tag="big", bufs=2)
                    for pr in range(4):
                        nc.tensor.matmul(
                            out=ps[:], lhsT=ctb_v[:, kk, pr, :, :],
                            rhs=t1p_t[:, 2 * pr:2 * pr + 2, :],
                            perf_mode=DR, start=(pr == 0), stop=(pr == 3),
                            skip_group_check=True)
                    msb = wp.tile([CHW, F2], dt.bfloat16, tag="msb", bufs=3)
                    nc.vector.tensor_tensor(out=msb[:], in0=ps[:],
                                            in1=h0own_v[:, k, :], op=OP.add)
                    nc.scalar.activation(hon1_v[:, k, :], msb[:], AF.Gelu)
                # stream this segment's prebuilt selection matrices
                nbs = segslots[s] // 128
                b0 = B0[SEG * s]
                nc.sync.dma_start(
                    out=sel_t[:, b0:b0 + nbs, :],
                    in_=selin[:, b0 * CHW:(b0 + nbs) * CHW].rearrange(
                        "p (b d) -> p b d", d=CHW))
                ag_piece(s, agin1, agout1)

            # ---------------- layers 2 and 3 ----------------
            cregs = {}
            for layer in (2, 3):
                table = agout1 if layer == 2 else agout2
                own = hon1_v if layer == 2 else hon2_v
                for s in range(NSEG):
                    gb = gbufs[s % 2]
                    ss_ = segslots[s]
                    ci = 0
                    while ci * 1024 < ss_:
                        w0 = ci * 1024
                        ni = min(1024, ss_ - w0)
                        nbw = ni // 128
                        if nbw not in cregs:
                            cregs[nbw] = nc.gpsimd.value_load(
                                cnts_t[0:1, nbw:nbw + 1])
                        i0 = (segbase[s] + w0) // 16
                        nc.gpsimd.dma_gather(
                            gb[:, w0 // 128:w0 // 128 + nbw, :], table[:],
                            idx_t[:, i0:i0 + ni // 16],
                            ni, cregs[nbw], F2)
                        ci += 1
                    for kk in range(SEG):
                        k = SEG * s + kk
                        nbk = nb[k]
                        b0 = B0[k]
                        g0 = B0[k] - B0[SEG * s]
                        ps = pp.tile([CHW, F2], dt.float32, tag="big", bufs=2)
                        j = 0
                        while j < nbk:
                            if j + 1 < nbk:
                                nc.tensor.matmul(
                                    out=ps[:],
                                    lhsT=sel_t[:, b0 + j:b0 + j + 2, :],
                                    rhs=gb[:, g0 + j:g0 + j + 2, :],
                                    perf_mode=DR, start=(j == 0),
                                    stop=(j + 2 >= nbk), skip_group_check=True)
                                j += 2
                            else:
                                nc.tensor.matmul(
                                    out=ps[:], lhsT=sel_t[:, b0 + j, :],
                                    rhs=gb[:, g0 + j, :],
                                    start=(j == 0), stop=True,
                                    skip_group_check=True)
                                j += 1
                        msb = wp.tile([CHW, F2], dt.bfloat16, tag="msb", bufs=3)
                        nc.vector.tensor_tensor(out=msb[:], in0=ps[:],
                                                in1=own[:, k, :], op=OP.add)
                        for jj in range(2):
                            tp = pp.tile([128, 128], dt.bfloat16, tag="tp", bufs=2)
                            nc.tensor.transpose(
                                tp[:], msb[:, jj * 128:(jj + 1) * 128], i128_t[:])
                            nc.vector.tensor_copy(
                                out=rhsT[:, jj, k * CHW:(k + 1) * CHW], in_=tp[:])
                    # node matmul + gelu for this segment
                    if layer == 2:
                        for jo in range(2):
                            psz = pp.tile([128, SEGN], dt.float32, tag="big",
                                          bufs=2)
                            nc.tensor.matmul(
                                out=psz[:], lhsT=w2_v[:, :, jo, :],
                                rhs=rhsT[:, :, s * SEGN:(s + 1) * SEGN],
                                perf_mode=DR, start=True, stop=True,
                                skip_group_check=True)
                            hf = wp.tile([128, SEGN], dt.bfloat16, tag="hf",
                                         bufs=2)
                            nc.scalar.activation(hf[:], psz[:], AF.Gelu,
                                                 bias=b2c_t[:, jo:jo + 1])
                            for tt in range(SEG):
                                tp2 = pp.tile([128, 128], dt.bfloat16,
                                              tag="tp", bufs=2)
                                nc.tensor.transpose(
                                    tp2[:], hf[:, tt * 128:(tt + 1) * 128],
                                    i128_t[:])
                                nc.vector.tensor_copy(
                                    out=hon2_v[:, SEG * s + tt,
                                               jo * 128:(jo + 1) * 128],
                                    in_=tp2[:])
                        state["hon_cur"] = hon2_v
                        ag_piece(s, agin2, agout2)
                    else:
                        psz = pp.tile([128, SEGN], dt.float32, tag="big", bufs=2)
                        nc.tensor.matmul(
                            out=psz[:], lhsT=w3_v[:, :, :],
                            rhs=rhsT[:, :, s * SEGN:(s + 1) * SEGN],
                            perf_mode=DR, start=True, stop=True,
                            skip_group_check=True)
                        nc.scalar.activation(hT3[:, s * SEGN:(s + 1) * SEGN],
                                             psz[:], AF.Gelu, bias=b3c_t[:, 0:1])
                        # attention tiles for this segment
                        psc = pp.tile([128, 16], dt.float32, tag="psc", bufs=1)
                        for tt in range(4):
                            t = 4 * s + tt
                            nc.tensor.matmul(
                                out=psc[:, tt * 4:(tt + 1) * 4],
                                lhsT=hT3[:, t * 128:(t + 1) * 128], rhs=qkf_t[:],
                                start=True, stop=True)
                            psv = pp.tile([128, 128], dt.float32, tag="att",
                                          bufs=2)
                            nc.tensor.matmul(
                                out=psv[:], lhsT=hT3[:, t * 128:(t + 1) * 128],
                                rhs=wvt_t[:], start=True, stop=True)
                            nc.vector.tensor_copy(out=vnm[:, t, :], in_=psv[:])
                        nc.scalar.activation(esc[:, s * 16:(s + 1) * 16],
                                             psc[:], AF.Exp)
                        if s % 2 == 1:
                            g = s // 2
                            psE = pp.tile([32, 1], dt.float32, tag="att", bufs=2)
                            nc.tensor.matmul(out=psE[:],
                                             lhsT=esc[:, g * 32:(g + 1) * 32],
                                             rhs=ones_t[:], start=True, stop=True)
                            s32 = wp.tile([32, 1], dt.float32, tag="s32", bufs=2)
                            nc.vector.tensor_copy(out=s32[:], in_=psE[:])
                            ps4 = pp.tile([4, 1], dt.float32, tag="att", bufs=2)
                            nc.tensor.matmul(out=ps4[:], lhsT=hsel_t[:],
                                             rhs=s32[:], start=True, stop=True)
                            sums4 = wp.tile([4, 1], dt.float32, tag="sums4",
                                            bufs=2)
                            nc.vector.tensor_tensor(out=sums4[:], in0=ps4[:],
                                                    in1=ecls_t[:], op=OP.add)
                            rr4 = wp.tile([4, 1], dt.float32, tag="rr4", bufs=2)
                            nc.vector.reciprocal(rr4[:], sums4[:])
                            psr = pp.tile([128, 1], dt.float32, tag="att", bufs=2)
                            nc.tensor.matmul(out=psr[:], lhsT=r4_t[:],
                                             rhs=rr4[:], start=True, stop=True)
                            rbc = wp.tile([128, 1], dt.float32, tag="rbc", bufs=2)
                            nc.vector.tensor_copy(out=rbc[:], in_=psr[:])
                            psg = pp.tile([128, 4], dt.float32, tag="att", bufs=2)
                            for t in range(8):
                                nc.tensor.matmul(
                                    out=psg[:], lhsT=vnm[:, 8 * g + t, :],
                                    rhs=esc[:, (8 * g + t) * 4:
                                            (8 * g + t + 1) * 4],
                                    start=(t == 0), stop=False,
                                    skip_group_check=True)
                            nc.tensor.matmul(out=psg[:], lhsT=vc4_t[:],
                                             rhs=e4_t[:], start=False, stop=True,
                                             skip_group_check=True)
                            tmp4 = wp.tile([128, 4], dt.float32, tag="tmp4",
                                           bufs=2)
                            nc.vector.tensor_tensor(out=tmp4[:], in0=psg[:],
                                                    in1=msel_t[:], op=OP.mult)
                            ctxv = wp.tile([128, 1], dt.float32, tag="ctxv",
                                           bufs=2)
                            nc.vector.reduce_sum(out=ctxv[:], in_=tmp4[:],
                                                 axis=mybir.AxisListType.X)
                            nc.vector.tensor_scalar(
                                out=ctx_all[:, g:g + 1], in0=ctxv[:],
                                scalar1=rbc[:], scalar2=bvt_t[:],
                                op0=OP.mult, op1=OP.add)

            # ---------------- output projection + LayerNorm ----------------
            psao = pp.tile([128, 4], dt.float32, tag="att", bufs=2)
            nc.tensor.matmul(out=psao[:], lhsT=wo_t[:], rhs=ctx_all[:],
                             start=True, stop=True)
            ysb = wp.tile([128, 4], dt.float32, tag="ysb")
            nc.vector.tensor_scalar(out=ysb[:], in0=psao[:],
                                    scalar1=ynb_t[:], scalar2=None, op0=OP.add)
            psy = pp.tile([4, 128], dt.float32, tag="att", bufs=2)
            nc.tensor.matmul(out=psy[:], lhsT=ysb[:], rhs=i128f_t[:],
                             is_transpose=True)
            yt = wp.tile([4, 128], dt.float32, tag="yt")
            nc.vector.tensor_copy(out=yt[:], in_=psy[:])
            mn = wp.tile([4, 1], dt.float32, tag="mn")
            nc.vector.reduce_sum(out=mn[:], in_=yt[:], axis=mybir.AxisListType.X)
            nc.vector.tensor_scalar(out=mn[:], in0=mn[:], scalar1=1.0 / D,
                                    scalar2=None, op0=OP.mult)
            xc = wp.tile([4, 128], dt.float32, tag="xc")
            nc.vector.tensor_scalar(out=xc[:], in0=yt[:], scalar1=mn[:],
                                    scalar2=None, op0=OP.subtract)
            sq = wp.tile([4, 128], dt.float32, tag="sq")
            ss = wp.tile([4, 1], dt.float32, tag="ss")
            nc.scalar.activation(sq[:], xc[:], AF.Square, accum_out=ss[:])
            sd = wp.tile([4, 1], dt.float32, tag="sd")
            nc.scalar.activation(sd[:], ss[:], AF.Sqrt, bias=eps_t[:],
                                 scale=1.0 / D)
            rr = wp.tile([4, 1], dt.float32, tag="rr")
            nc.vector.reciprocal(rr[:], sd[:])
            yn = wp.tile([4, 128], dt.float32, tag="yn")
            nc.vector.tensor_scalar(out=yn[:], in0=xc[:], scalar1=rr[:],
                                    scalar2=None, op0=OP.mult)
            nc.vector.tensor_tensor(out=yn[:], in0=yn[:], in1=lng_t[:],
                                    op=OP.mult)
            nc.vector.tensor_tensor(out=yn[:], in0=yn[:], in1=lnb_t[:],
                                    op=OP.add)
            nc.sync.dma_start(out=y_out[:], in_=yn[:])

    nc.compile()
    _prog_cache[key] = nc
    return nc


def kernel(**inputs):
    from concourse.bass_utils import run_bass_kernel_spmd
    in_maps, P = _host_prep(inputs)
    nc = _build_program("hw", P)
    res = run_bass_kernel_spmd(nc, in_maps, core_ids=list(range(NCORES)))
    y = np.concatenate([res.results[c]["y"] for c in range(NCORES)], axis=0)
    return np.ascontiguousarray(y.astype(np.float32))


# revision 7
# speedup vs baseline: 1.7609x; 1.0226x over previous
"""Trainium2 Bass kernel for nn_GTShapelet (GIN stack + CLS-query MHA).

Self-contained: builds the Bass/Tile program, shards inputs across 8
NeuronCores (data-parallel over destination-node ranges; graphs 4c..4c+3
on core c), runs via run_bass_kernel_spmd, and reassembles the full
[32, 128] output.

Design (vs. the 590us baseline):
  - fp8(e4m3) tables wherever the DMA/PE cost rewards it: gather tables
    (h1, h2), ct count-matrix, T1, W2/W3, selection matrices.  Gather
    rows are 256B (the dma_gather minimum granularity).
  - Edge slots are compacted per 128-dst chunk (128-aligned), and
    dma_gather calls are 1024-index windows decoupled from chunk
    boundaries (the gather ucode caps at 1024 idxs/call).
  - DoubleRow fp8 matmuls (0.5 cyc/row) for the ct and selection
    segment-sum groups and the node matmuls.
  - Selection matrices are built once on DVE (overlapped with layer 1)
    and reused by layers 2 and 3 (identical slotting).
  - The "h + msg" own-row add runs on DVE against the chunk PSUM from
    node-major own tables; biases fold into the own table (L1) or the
    feature-major gelu (L2/L3).
  - The inter-layer AllGather is split into 8 per-segment pieces that
    overlap the producing layer's compute.
  - Attention: K-projection folded into the query on the host
    (scoresT = hT3 @ (Wk qblk); the per-head key-bias constant cancels
    in softmax), scores/exp/V-projection computed node-major per
    segment interleaved with layer 3; no transposes in the tail.
"""

import sys

if "/opt/trn_rl_repo" not in sys.path:
    sys.path.insert(0, "/opt/trn_rl_repo")

import numpy as np
import ml_dtypes

# ---- problem constants (hardcoded per spec) ----
B, N, E, D = 32, 1024, 524288, 128
H, HD = 4, 32
F2 = 2 * D                     # 256
NCORES = 8
NPC = B * N // NCORES          # 4096 nodes per core
GPC = B // NCORES              # 4 graphs per core
CHW = 128                      # dst-chunk width (nodes)
NCH = NPC // CHW               # 32 chunks per core
SEG = 4                        # chunks per segment
NSEG = NCH // SEG              # 8 segments per core
SEGN = SEG * CHW               # 512 nodes per segment
BF16 = ml_dtypes.bfloat16
FP8 = ml_dtypes.float8_e4m3

_prog_cache = {}


def _wrap16(arr):
    """slot i -> [i % 16, i // 16], replicated into partitions 16..31.

    CoreSim's gather ucode reads partitions 0..15; the deployed HW ucode
    reads 16..31 -- fill both so either path sees the indices.
    """
    n = arr.shape[0]
    out = np.zeros((128, n // 16), np.int16)
    w = arr.reshape(n // 16, 16).T.astype(np.int16)
    out[0:16] = w
    out[16:32] = w
    return out


def _host_prep(inputs):
    node_ids = np.asarray(inputs["node_ids"]).astype(np.int64)
    src = np.asarray(inputs["src"]).astype(np.int64)
    dst = np.asarray(inputs["dst"]).astype(np.int64)
    pad_mask = np.asarray(inputs["pad_mask"])
    ew = np.asarray(inputs["edge_weight"]).astype(np.float64)
    embed = np.asarray(inputs["embed_table"]).astype(np.float64)
    W1 = np.asarray(inputs["W1"]).astype(np.float64)
    b1 = np.asarray(inputs["b1"]).astype(np.float32)
    W2 = np.asarray(inputs["W2"]).astype(np.float32)
    b2 = np.asarray(inputs["b2"]).astype(np.float32)
    W3 = np.asarray(inputs["W3"]).astype(np.float32)
    b3 = np.asarray(inputs["b3"]).astype(np.float32)
    ipw = np.asarray(inputs["in_proj_w"]).astype(np.float64)
    ipb = np.asarray(inputs["in_proj_b"]).astype(np.float64)
    ow = np.asarray(inputs["out_w"]).astype(np.float32)
    ob = np.asarray(inputs["out_b"]).astype(np.float32)
    cls = np.asarray(inputs["cls_embedding"]).astype(np.float64).reshape(D)
    ln_g = np.asarray(inputs["ln_g"]).astype(np.float32)
    ln_b = np.asarray(inputs["ln_b"]).astype(np.float32)

    assert not pad_mask.any(), "kernel compiled for all-False pad_mask"

    # ---- shared (replicated) constants ----
    T1 = (embed @ W1).astype(np.float32)                 # [1024, 256]
    t1p = T1.reshape(N // 128, 128, F2).transpose(1, 0, 2).astype(FP8)

    Wq, Wk, Wv = ipw[:, :D], ipw[:, D:2 * D], ipw[:, 2 * D:]
    bq, bk_, bv_ = ipb[:D], ipb[D:2 * D], ipb[2 * D:]
    q_cls = (cls @ Wq + bq) / np.sqrt(HD)                # [128]
    qblk = np.zeros((D, H))
    for h in range(H):
        qblk[h * HD:(h + 1) * HD, h] = q_cls[h * HD:(h + 1) * HD]
    qkf = (Wk @ qblk).astype(np.float32)                 # [128, 4]
    bkq = np.array([bk_ @ qblk[:, h] for h in range(H)])
    k_cls = cls @ Wk + bk_
    s_cls = np.array([q_cls[h * HD:(h + 1) * HD] @ k_cls[h * HD:(h + 1) * HD]
                      for h in range(H)])
    e_cls = np.exp(s_cls - bkq)                          # device scores omit bkq
    v_cls_nb = cls @ Wv                                  # bias added post-softmax
    vc4 = np.zeros((4, 128), np.float32)
    for h in range(H):
        vc4[h, h * HD:(h + 1) * HD] = v_cls_nb[h * HD:(h + 1) * HD]
    e4 = np.diag(e_cls).astype(np.float32)
    msel = np.zeros((128, 4), np.float32)
    r4 = np.zeros((4, 128), np.float32)
    for h in range(H):
        msel[h * HD:(h + 1) * HD, h] = 1.0
        r4[h, h * HD:(h + 1) * HD] = 1.0
    hsel = np.zeros((32, 4), np.float32)
    for j in range(32):
        hsel[j, j % 4] = 1.0

    w2dr = W2.reshape(2, 128, 2, 128).transpose(1, 0, 2, 3).astype(FP8)
    w3dr = W3.reshape(2, 128, 128).transpose(1, 0, 2).astype(FP8)

    # ---- edge slotting (core-uniform: program is SPMD) ----
    ew32 = ew.astype(np.float32)
    order_all = np.argsort(dst, kind='stable')
    dst_sorted = dst[order_all]
    chunk_starts = np.searchsorted(dst_sorted, np.arange(0, B * N + 1, CHW))
    cnt_all = np.diff(chunk_starts).reshape(NCORES, NCH)      # [core, chunk]
    nb_uni = np.maximum(1, -(-cnt_all.max(0) // 128)).astype(np.int64)  # [32]
    B0 = np.concatenate([[0], np.cumsum(nb_uni)]).astype(np.int64)      # [33]
    NBT = int(B0[-1])
    segslots = [int(128 * (B0[SEG * (s + 1)] - B0[SEG * s]))
                for s in range(NSEG)]
    segbase = np.concatenate([[0], np.cumsum(segslots)]).astype(np.int64)
    CAPT = int(segbase[-1])
    SEGBMAX = max(sl // 128 for sl in segslots)

    # agout row permutation: node n -> row (seg<<12 | core<<9 | offset)
    nvec = np.arange(B * N, dtype=np.int64)
    agrow = ((nvec & 4095) >> 9 << 12) | (nvec >> 12 << 9) | (nvec & 511)

    P = dict(nb=tuple(int(x) for x in nb_uni), NBT=NBT, CAPT=CAPT,
             segslots=tuple(segslots), SEGBMAX=SEGBMAX)

    shared = {
        "t1p": t1p,
        "w2dr": w2dr.reshape(128, 2 * 2 * 128),
        "w3dr": w3dr.reshape(128, 2 * 128),
        "b2c": b2.reshape(2, 128).T.copy(),
        "b3c": b3.reshape(128, 1).copy(),
        "wvt": Wv.astype(BF16),
        "qkf": qkf.astype(BF16),
        "i128": np.eye(128, dtype=np.float32).astype(BF16),
        "i128f": np.eye(128, dtype=np.float32),
        "ones128": np.ones((128, 1), np.float32).astype(BF16),
        "hsel": hsel,
        "r4": r4,
        "msel": msel,
        "vc4": vc4.astype(BF16),
        "e4": e4.astype(BF16),
        "ecls": e_cls.astype(np.float32).reshape(4, 1),
        "bvt": bv_.astype(np.float32).reshape(128, 1),
        "ynb": (cls + ob).astype(np.float32).reshape(128, 1),
        "wo": ow.astype(BF16),
        "lng": np.tile(ln_g, (4, 1)),
        "lnb": np.tile(ln_b, (4, 1)),
        "eps": np.full((4, 1), 1e-5, np.float32),
        "cnts": (np.arange(9, dtype=np.int32) * 128).reshape(1, 9),
    }

    T1b = T1 + b1[None, :]                                # b1 folded into own rows
    in_maps = []
    for c in range(NCORES):
        g_idx = np.zeros(CAPT, np.int64)
        selh = np.zeros((NBT * 128, CHW), np.float32)
        for k in range(NCH):
            gk = c * NCH + k
            a, bnd = chunk_starts[gk], chunk_starts[gk + 1]
            cnt = bnd - a
            assert cnt <= 128 * nb_uni[k]
            e = order_all[a:bnd]
            base = 128 * B0[k]
            g_idx[base:base + cnt] = agrow[src[e]]
            dl = (dst[e] - (c * NPC + k * CHW)).astype(np.int64)
            selh[np.arange(base, base + cnt), dl] = ew32[e]
        eidx = order_all[chunk_starts[c * NCH]:chunk_starts[(c + 1) * NCH]]
        ids_e = node_ids[src[eidx]]
        dl_e = dst[eidx] - c * NPC
        Cf = np.bincount(dl_e * N + ids_e, weights=ew[eidx],
                         minlength=NPC * N).reshape(NPC, N).astype(np.float32)
        # ct[p, k, pr, j, d] = Cf[k*128+d, (2pr+j)*128+p]
        ct = Cf.reshape(NCH, CHW, 8, 128).transpose(3, 0, 2, 1).astype(FP8)
        nids_own = node_ids[c * NPC:(c + 1) * NPC]
        # h0own node-major [128, 32, 256], bias b1 folded in
        h0own = T1b[nids_own].reshape(NCH, CHW, F2).transpose(1, 0, 2).astype(FP8)
        m = dict(shared)
        m.update({
            "ct": np.ascontiguousarray(ct).reshape(128, NCH * 8 * CHW),
            "h0own": np.ascontiguousarray(h0own).reshape(128, NCH * F2),
            "idx12": _wrap16(g_idx),
            "selin": np.ascontiguousarray(
                selh.reshape(NBT, 128, CHW).transpose(1, 0, 2)
            ).astype(FP8).reshape(128, NBT * CHW),
        })
        in_maps.append(m)
    return in_maps, P


def _build_program(variant, P):
    key = (variant, P['nb'], P['segslots'])
    if key in _prog_cache:
        return _prog_cache[key]
    import concourse.bacc as bacc
    import concourse.tile as tile
    import concourse.mybir as mybir

    dt = mybir.dt
    AF = mybir.ActivationFunctionType
    OP = mybir.AluOpType
    DR = mybir.MatmulPerfMode.DoubleRow

    nb = P['nb']
    NBT = P['NBT']
    CAPT = P['CAPT']
    segslots = P['segslots']
    SEGBMAX = P['SEGBMAX']
    B0 = [0]
    for x in nb:
        B0.append(B0[-1] + x)
    segbase = [0]
    for sl in segslots:
        segbase.append(segbase[-1] + sl)

    nc = bacc.Bacc("TRN2", target_bir_lowering=False, debug=False,
                   num_devices=(1 if variant == "sim1" else NCORES))

    def din(name, shape, dtype):
        return nc.dram_tensor(name, shape, dtype, kind="ExternalInput")

    t1p = din("t1p", [128, 8, F2], dt.float8e4)
    ct = din("ct", [128, NCH * 8 * CHW], dt.float8e4)
    h0own = din("h0own", [128, NCH * F2], dt.float8e4)
    idx12 = din("idx12", [128, CAPT // 16], dt.int16)
    cnts = din("cnts", [1, 9], dt.int32)
    selin = din("selin", [128, NBT * CHW], dt.float8e4)
    w2dr = din("w2dr", [128, 512], dt.float8e4)
    w3dr = din("w3dr", [128, 256], dt.float8e4)
    b2c = din("b2c", [128, 2], dt.float32)
    b3c = din("b3c", [128, 1], dt.float32)
    wvt = din("wvt", [128, 128], dt.bfloat16)
    qkf = din("qkf", [128, 4], dt.bfloat16)
    i128 = din("i128", [128, 128], dt.bfloat16)
    i128f = din("i128f", [128, 128], dt.float32)
    ones128 = din("ones128", [128, 1], dt.bfloat16)
    hsel = din("hsel", [32, 4], dt.float32)
    r4 = din("r4", [4, 128], dt.float32)
    msel = din("msel", [128, 4], dt.float32)
    vc4 = din("vc4", [4, 128], dt.bfloat16)
    e4 = din("e4", [4, 4], dt.bfloat16)
    ecls = din("ecls", [4, 1], dt.float32)
    bvt = din("bvt", [128, 1], dt.float32)
    ynb = din("ynb", [128, 1], dt.float32)
    wo = din("wo", [128, 128], dt.bfloat16)
    lng = din("lng", [4, 128], dt.float32)
    lnb = din("lnb", [4, 128], dt.float32)
    eps = din("eps", [4, 1], dt.float32)
    y_out = nc.dram_tensor("y", [GPC, D], dt.float32, kind="ExternalOutput")

    with tile.TileContext(nc) as tc:
        from concourse.library_config import mlp
        nc.gpsimd.load_library(mlp)
        with tc.tile_pool(name="const", bufs=1) as cp, \
             tc.tile_pool(name="res", bufs=1) as rp, \
             tc.tile_pool(name="work", bufs=1) as wp, \
             tc.tile_pool(name="ps", bufs=1, space="PSUM") as pp, \
             tc.tile_pool(name="dram", bufs=2, space="DRAM") as dram:

            def cload(ap, shape, dtype):
                t = cp.tile(shape, dtype, name=f"c_{ap.name}")
                nc.sync.dma_start(out=t[:], in_=ap[:])
                return t

            t1p_t = cload(t1p, [128, 8, F2], dt.float8e4)
            ctbufs = [wp.tile([128, SEG * 8 * CHW], dt.float8e4, tag=f"ctb{i}",
                              name=f"ctb{i}") for i in range(2)]
            nc.sync.dma_start(out=ctbufs[0][:], in_=ct[:, 0:SEG * 8 * CHW])
            h0own_t = cload(h0own, [128, NCH * F2], dt.float8e4)
            h0own_v = h0own_t[:].rearrange("p (k f) -> p k f", f=F2)
            idx_t = cload(idx12, [128, CAPT // 16], dt.int16)
            cnts_t = cload(cnts, [1, 9], dt.int32)
            i128_t = cload(i128, [128, 128], dt.bfloat16)

            sel_t = rp.tile([128, NBT, CHW], dt.float8e4, name="sel")
            hon1 = rp.tile([128, NCH, F2], dt.float8e4, name="hon1")
            hon2 = rp.tile([128, NCH, F2], dt.float8e4, name="hon2")
            rhsT = rp.tile([128, 2, NPC], dt.float8e4, name="rhsT")
            hT3 = rp.tile([128, NPC], dt.bfloat16, name="hT3")
            vnm = rp.tile([128, NPC // 128, 128], dt.bfloat16, name="vnm")
            esc = rp.tile([128, 128], dt.bfloat16, name="esc")
            ctx_all = rp.tile([128, 4], dt.bfloat16, name="ctx_all")
            gbufs = [wp.tile([128, SEGBMAX, F2], dt.float8e4, tag=f"gb{i}",
                             name=f"gbuf{i}") for i in range(2)]

            agin1 = dram.tile([NPC, F2], dt.float8e4, tag="agin1")
            agout1 = dram.tile([B * N, F2], dt.float8e4, tag="agout1")
            agin2 = dram.tile([NPC, F2], dt.float8e4, tag="agin2")
            agout2 = dram.tile([B * N, F2], dt.float8e4, tag="agout2")

            hon1_v = hon1[:]
            hon2_v = hon2[:]
            state = {"hon_cur": hon1_v}

            def ag_piece(s, agin, agout):
                # send this core's segment-s rows, receive everyone's
                agin_v = agin.rearrange("(g tt p) f -> p g tt f", tt=SEG, p=CHW)
                nc.sync.dma_start(out=agin_v[:, s, :, :],
                                  in_=state["hon_cur"][:, SEG * s:SEG * (s + 1), :])
                if variant == "sim1":
                    nc.sync.dma_start(
                        out=agout[s * NPC:(s + 1) * NPC, :].rearrange(
                            "(c n) f -> c n f", c=NCORES),
                        in_=agin[s * SEGN:(s + 1) * SEGN, :].unsqueeze(0)
                            .broadcast_to([NCORES, SEGN, F2]))
                else:
                    nc.gpsimd.collective_compute(
                        "AllGather", mybir.AluOpType.bypass,
                        replica_groups=[list(range(NCORES))],
                        ins=[agin[s * SEGN:(s + 1) * SEGN, :].opt()],
                        outs=[agout[s * NPC:(s + 1) * NPC, :].opt()])

            # ---------------- layer 1 (ct x T1) ----------------
            for s in range(NSEG):
                if s + 1 < NSEG:
                    nc.sync.dma_start(
                        out=ctbufs[(s + 1) % 2][:],
                        in_=ct[:, (s + 1) * SEG * 8 * CHW:(s + 2) * SEG * 8 * CHW])
                ctb_v = ctbufs[s % 2][:].rearrange(
                    "p (kk pr j d) -> p kk pr j d", pr=4, j=2, d=CHW)
                for kk in range(SEG):
                    k = SEG * s + kk
                    ps = pp.tile([CHW, F2], dt.float32, tag="big", bufs=2)
                    for pr in range(4):
                        nc.tensor.matmul(
                            out=ps[:], lhsT=ctb_v[:, kk, pr, :, :],
                            rhs=t1p_t[:, 2 * pr:2 * pr + 2, :],
                            perf_mode=DR, start=(pr == 0), stop=(pr == 3),
                            skip_group_check=True)
                    msb = wp.tile([CHW, F2], dt.bfloat16, tag="msb", bufs=3)
                    nc.vector.tensor_tensor(out=msb[:], in0=ps[:],
                                            in1=h0own_v[:, k, :], op=OP.add)
                    nc.scalar.activation(hon1_v[:, k, :], msb[:], AF.Gelu)
                # stream this segment's prebuilt selection matrices
                nbs = segslots[s] // 128
                b0 = B0[SEG * s]
                nc.sync.dma_start(
                    out=sel_t[:, b0:b0 + nbs, :],
                    in_=selin[:, b0 * CHW:(b0 + nbs) * CHW].rearrange(
                        "p (b d) -> p b d", d=CHW))
                ag_piece(s, agin1, agout1)

            # small constants: issued after L1's streams so they don't
            # block the prologue on HWDGE; they land in the L1->L2 gap
            w2_t = cload(w2dr, [128, 512], dt.float8e4)
            w2_v = w2_t[:].rearrange("p (t jo o) -> p t jo o", t=2, jo=2)
            w3_t = cload(w3dr, [128, 256], dt.float8e4)
            w3_v = w3_t[:].rearrange("p (t o) -> p t o", t=2)
            b2c_t = cload(b2c, [128, 2], dt.float32)
            b3c_t = cload(b3c, [128, 1], dt.float32)
            wvt_t = cload(wvt, [128, 128], dt.bfloat16)
            qkf_t = cload(qkf, [128, 4], dt.bfloat16)
            i128f_t = cload(i128f, [128, 128], dt.float32)
            ones_t = cload(ones128, [128, 1], dt.bfloat16)
            hsel_t = cload(hsel, [32, 4], dt.float32)
            r4_t = cload(r4, [4, 128], dt.float32)
            msel_t = cload(msel, [128, 4], dt.float32)
            vc4_t = cload(vc4, [4, 128], dt.bfloat16)
            e4_t = cload(e4, [4, 4], dt.bfloat16)
            ecls_t = cload(ecls, [4, 1], dt.float32)
            bvt_t = cload(bvt, [128, 1], dt.float32)
            ynb_t = cload(ynb, [128, 1], dt.float32)
            wo_t = cload(wo, [128, 128], dt.bfloat16)
            lng_t = cload(lng, [4, 128], dt.float32)
            lnb_t = cload(lnb, [4, 128], dt.float32)
            eps_t = cload(eps, [4, 1], dt.float32)

            # ---------------- layers 2 and 3 ----------------
            cregs = {}
            for layer in (2, 3):
                table = agout1 if layer == 2 else agout2
                own = hon1_v if layer == 2 else hon2_v
                for s in range(NSEG):
                    gb = gbufs[s % 2]
                    ss_ = segslots[s]
                    ci = 0
                    while ci * 1024 < ss_:
                        w0 = ci * 1024
                        ni = min(1024, ss_ - w0)
                        nbw = ni // 128
                        if nbw not in cregs:
                            cregs[nbw] = nc.gpsimd.value_load(
                                cnts_t[0:1, nbw:nbw + 1])
                        i0 = (segbase[s] + w0) // 16
                        nc.gpsimd.dma_gather(
                            gb[:, w0 // 128:w0 // 128 + nbw, :], table[:],
                            idx_t[:, i0:i0 + ni // 16],
                            ni, cregs[nbw], F2)
                        ci += 1
                    for kk in range(SEG):
                        k = SEG * s + kk
                        nbk = nb[k]
                        b0 = B0[k]
                        g0 = B0[k] - B0[SEG * s]
                        ps = pp.tile([CHW, F2], dt.float32, tag="big", bufs=2)
                        j = 0
                        while j < nbk:
                            if j + 1 < nbk:
                                nc.tensor.matmul(
                                    out=ps[:],
                                    lhsT=sel_t[:, b0 + j:b0 + j + 2, :],
                                    rhs=gb[:, g0 + j:g0 + j + 2, :],
                                    perf_mode=DR, start=(j == 0),
                                    stop=(j + 2 >= nbk), skip_group_check=True)
                                j += 2
                            else:
                                nc.tensor.matmul(
                                    out=ps[:], lhsT=sel_t[:, b0 + j, :],
                                    rhs=gb[:, g0 + j, :],
                                    start=(j == 0), stop=True,
                                    skip_group_check=True)
                                j += 1
                        msb = wp.tile([CHW, F2], dt.bfloat16, tag="msb", bufs=3)
                        nc.vector.tensor_tensor(out=msb[:], in0=ps[:],
                                                in1=own[:, k, :], op=OP.add)
                        for jj in range(2):
                            tp = pp.tile([128, 128], dt.bfloat16, tag="tp", bufs=2)
                            nc.tensor.transpose(
                                tp[:], msb[:, jj * 128:(jj + 1) * 128], i128_t[:])
                            nc.vector.tensor_copy(
                                out=rhsT[:, jj, k * CHW:(k + 1) * CHW], in_=tp[:])
                    # per-chunk node matmul + gelu (short tail chain)
                    if layer == 2:
                        for kk in range(SEG):
                            k = SEG * s + kk
                            for jo in range(2):
                                psz = pp.tile([128, 128], dt.float32,
                                              tag="tp", bufs=2)
                                nc.tensor.matmul(
                                    out=psz[:], lhsT=w2_v[:, :, jo, :],
                                    rhs=rhsT[:, :, k * CHW:(k + 1) * CHW],
                                    perf_mode=DR, start=True, stop=True,
                                    skip_group_check=True)
                                hfc = wp.tile([128, 128], dt.bfloat16,
                                              tag="hf", bufs=3)
                                nc.scalar.activation(hfc[:], psz[:], AF.Gelu,
                                                     bias=b2c_t[:, jo:jo + 1])
                                tp2 = pp.tile([128, 128], dt.bfloat16,
                                              tag="tp", bufs=2)
                                nc.tensor.transpose(tp2[:], hfc[:], i128_t[:])
                                nc.vector.tensor_copy(
                                    out=hon2_v[:, k, jo * 128:(jo + 1) * 128],
                                    in_=tp2[:])
                        state["hon_cur"] = hon2_v
                        ag_piece(s, agin2, agout2)
                    else:
                        psc = pp.tile([128, 16], dt.float32, tag="psc", bufs=1)
                        for kk in range(SEG):
                            k = SEG * s + kk
                            psz = pp.tile([128, 128], dt.float32,
                                          tag="tp", bufs=2)
                            nc.tensor.matmul(
                                out=psz[:], lhsT=w3_v[:, :, :],
                                rhs=rhsT[:, :, k * CHW:(k + 1) * CHW],
                                perf_mode=DR, start=True, stop=True,
                                skip_group_check=True)
                            nc.scalar.activation(
                                hT3[:, k * CHW:(k + 1) * CHW],
                                psz[:], AF.Gelu, bias=b3c_t[:, 0:1])
                            # attention tile for this chunk (tile t == k)
                            nc.tensor.matmul(
                                out=psc[:, kk * 4:(kk + 1) * 4],
                                lhsT=hT3[:, k * 128:(k + 1) * 128], rhs=qkf_t[:],
                                start=True, stop=True)
                            psv = pp.tile([128, 128], dt.float32, tag="att",
                                          bufs=2)
                            nc.tensor.matmul(
                                out=psv[:], lhsT=hT3[:, k * 128:(k + 1) * 128],
                                rhs=wvt_t[:], start=True, stop=True)
                            nc.vector.tensor_copy(out=vnm[:, k, :], in_=psv[:])
                        nc.scalar.activation(esc[:, s * 16:(s + 1) * 16],
                                             psc[:], AF.Exp)
                        if s % 2 == 1:
                            g = s // 2
                            psE = pp.tile([32, 1], dt.float32, tag="att", bufs=2)
                            nc.tensor.matmul(out=psE[:],
                                             lhsT=esc[:, g * 32:(g + 1) * 32],
                                             rhs=ones_t[:], start=True, stop=True)
                            s32 = wp.tile([32, 1], dt.float32, tag="s32", bufs=2)
                            nc.vector.tensor_copy(out=s32[:], in_=psE[:])
                            ps4 = pp.tile([4, 1], dt.float32, tag="att", bufs=2)
                            nc.tensor.matmul(out=ps4[:], lhsT=hsel_t[:],
                                             rhs=s32[:], start=True, stop=True)
                            sums4 = wp.tile([4, 1], dt.float32, tag="sums4",
                                            bufs=2)
                            nc.vector.tensor_tensor(out=sums4[:], in0=ps4[:],
                                                    in1=ecls_t[:], op=OP.add)
                            rr4 = wp.tile([4, 1], dt.float32, tag="rr4", bufs=2)
                            nc.vector.reciprocal(rr4[:], sums4[:])
                            psr = pp.tile([128, 1], dt.float32, tag="att", bufs=2)
                            nc.tensor.matmul(out=psr[:], lhsT=r4_t[:],
                                             rhs=rr4[:], start=True, stop=True)
                            rbc = wp.tile([128, 1], dt.float32, tag="rbc", bufs=2)
                            nc.vector.tensor_copy(out=rbc[:], in_=psr[:])
                            psg = pp.tile([128, 4], dt.float32, tag="att", bufs=2)
                            for t in range(8):
                                nc.tensor.matmul(
                                    out=psg[:], lhsT=vnm[:, 8 * g + t, :],
                                    rhs=esc[:, (8 * g + t) * 4:
                                            (8 * g + t + 1) * 4],
                                    start=(t == 0), stop=False,
                                    skip_group_check=True)
                            nc.tensor.matmul(out=psg[:], lhsT=vc4_t[:],
                                             rhs=e4_t[:], start=False, stop=True,
                                             skip_group_check=True)
                            tmp4 = wp.tile([128, 4], dt.float32, tag="tmp4",
                                           bufs=2)
                            nc.vector.tensor_tensor(out=tmp4[:], in0=psg[:],
                                                    in1=msel_t[:], op=OP.mult)
                            ctxv = wp.tile([128, 1], dt.float32, tag="ctxv",
                                           bufs=2)
                            nc.vector.reduce_sum(out=ctxv[:], in_=tmp4[:],
                                                 axis=mybir.AxisListType.X)
                            nc.vector.tensor_scalar(
                                out=ctx_all[:, g:g + 1], in0=ctxv[:],
                                scalar1=rbc[:], scalar2=bvt_t[:],
                                op0=OP.mult, op1=OP.add)

            # ---------------- output projection + LayerNorm ----------------
            psao = pp.tile([128, 4], dt.float32, tag="att", bufs=2)
            nc.tensor.matmul(out=psao[:], lhsT=wo_t[:], rhs=ctx_all[:],
                             start=True, stop=True)
            ysb = wp.tile([128, 4], dt.float32, tag="ysb")
            nc.vector.tensor_scalar(out=ysb[:], in0=psao[:],
                                    scalar1=ynb_t[:], scalar2=None, op0=OP.add)
            psy = pp.tile([4, 128], dt.float32, tag="att", bufs=2)
            nc.tensor.matmul(out=psy[:], lhsT=ysb[:], rhs=i128f_t[:],
                             is_transpose=True)
            yt = wp.tile([4, 128], dt.float32, tag="yt")
            nc.vector.tensor_copy(out=yt[:], in_=psy[:])
            mn = wp.tile([4, 1], dt.float32, tag="mn")
            nc.vector.reduce_sum(out=mn[:], in_=yt[:], axis=mybir.AxisListType.X)
            nc.vector.tensor_scalar(out=mn[:], in0=mn[:], scalar1=1.0 / D,
                                    scalar2=None, op0=OP.mult)
            xc = wp.tile([4, 128], dt.float32, tag="xc")
            nc.vector.tensor_scalar(out=xc[:], in0=yt[:], scalar1=mn[:],
                                    scalar2=None, op0=OP.subtract)
            sq = wp.tile([4, 128], dt.float32, tag="sq")
            ss = wp.tile([4, 1], dt.float32, tag="ss")
            nc.scalar.activation(sq[:], xc[:], AF.Square, accum_out=ss[:])
            sd = wp.tile([4, 1], dt.float32, tag="sd")
            nc.scalar.activation(sd[:], ss[:], AF.Sqrt, bias=eps_t[:],
                                 scale=1.0 / D)
            rr = wp.tile([4, 1], dt.float32, tag="rr")
            nc.vector.reciprocal(rr[:], sd[:])
            yn = wp.tile([4, 128], dt.float32, tag="yn")
            nc.vector.tensor_scalar(out=yn[:], in0=xc[:], scalar1=rr[:],
                                    scalar2=None, op0=OP.mult)
            nc.vector.tensor_tensor(out=yn[:], in0=yn[:], in1=lng_t[:],
                                    op=OP.mult)
            nc.vector.tensor_tensor(out=yn[:], in0=yn[:], in1=lnb_t[:],
                                    op=OP.add)
            nc.sync.dma_start(out=y_out[:], in_=yn[:])

    nc.compile()
    _prog_cache[key] = nc
    return nc


def kernel(**inputs):
    from concourse.bass_utils import run_bass_kernel_spmd
    in_maps, P = _host_prep(inputs)
    nc = _build_program("hw", P)
    res = run_bass_kernel_spmd(nc, in_maps, core_ids=list(range(NCORES)))
    y = np.concatenate([res.results[c]["y"] for c in range(NCORES)], axis=0)
    return np.ascontiguousarray(y.astype(np.float32))
